# revision 28
# baseline (speedup 1.0000x reference)
"""Trainium2 Bass kernel for nn_CausalAttention_84018150244353.

kernel(**inputs) takes the FULL unsharded inputs (as in reference
setup_inputs) and returns the full (2, 2048, 2048) float32 output.

Sharding: 8 NeuronCores = 2 batches x 4 head-slots. Heads are grouped
into 4 work classes by ALiBi slope (large slopes attend only a short
window, so distant key chunks are dropped); each core gets one head of
each class so all cores run the identical program with balanced work:
  slot 0 (heads 0-3):   5 key chunks per query chunk
  slot 1 (heads 4-7):   6 key chunks
  slot 2 (heads 8-11):  9 key chunks
  slot 3 (heads 12-15): full causal
Core (b, s) handles batch b and heads {s, s+4, s+8, s+12}.

Per-core program, software-pipelined so the PE never waits on the
Scalar engine's exp chain:
  proj(0) runs standalone; thereafter the q/k/v projection chains for
  chunk c+1 and the output-projection chains for chunk c-1 are pulled
  as PE "filler" work between attention tiles of chunk c (the exp on
  Scalar is the per-tile rate limiter; two 512-col filler matmuls per
  tile cover the deficit). The exp-sum accumulator (pacc) and the
  PSUM->SBUF copies run on the Vector engine (DVE).
  Host sums the 4 head-slot partials, adds bo + bv @ Wo.
"""
import math
import os
import sys
import time
from collections import deque

sys.path.insert(0, "/opt/trn_rl_repo")

import numpy as np
import jax

jax.config.update("jax_compilation_cache_dir",
                  os.environ.get("JAX_NEFF_CACHE", "/tmp/jax_neff_cache"))
jax.config.update("jax_persistent_cache_min_compile_time_secs", 0.0)
jax.config.update("jax_persistent_cache_min_entry_size_bytes", -1)

from jax.sharding import Mesh, PartitionSpec
from jax.experimental.shard_map import shard_map

import concourse.bass as bass
import concourse.mybir as mybir
import concourse.tile as tile
from concourse import bacc
from concourse import bass2jax
from concourse.bass2jax import _bass_exec_p, install_neuronx_cc_hook

f32 = mybir.dt.float32
r32 = mybir.dt.float32r
bf16 = mybir.dt.bfloat16
Exp = mybir.ActivationFunctionType.Exp
Ident = mybir.ActivationFunctionType.Identity
AddOp = mybir.AluOpType.add

T = 2048
EMB = 2048
HG = 512          # columns per core (4 heads x 128)
HD = 128
NH = 4            # heads per core
NQ = 4            # T quarters
QT = T // NQ      # 512
NE = EMB // 128   # 16 contraction chunks
NJ = T // 128     # 16 key chunks
KS = (5, 6, 9, 16)    # kept key chunks per head slot (ALiBi cutoff)
LOOK = 3


def kept_range(c: int, s: int):
    return range(max(0, 4 * c + 4 - KS[s]), 4 * c + 4)


# Max ALiBi slope of the heads a slot can hold (slot s holds heads
# {s, s+4, s+8, s+12}; classes are h//4).
_SLOT_MAX_SLOPE = (2.0 ** -0.5, 2.0 ** -2.5, 2.0 ** -4.5, 2.0 ** -6.5)


def needs_msh(c: int, s: int) -> bool:
    """Whether exp() needs the rank-1 per-query stabilization shift.
    With the bias re-anchored at the last query of chunk c, the exponent
    floor is -slope * max-key-depth; fp32 exp stays in normal range
    (given |s| <~ 8) as long as that is above ~-70."""
    depth = min(512 * c + 511, 511 + 128 * (KS[s] - 4))
    return _SLOT_MAX_SLOPE[s] * depth > 70.0


def build_program(reps: int = 1, sim_safe: bool = False):
    """sim_safe=True keeps start-matmuls full-width so exec-mode CoreSim's
    PSUM pending-zero bookkeeping holds (hardware is fine either way)."""
    nc = bacc.Bacc("TRN2", target_bir_lowering=False, debug=False,
                   enable_asserts=False, num_devices=8)

    # x and w are tile-packed host-side so every load is fully
    # contiguous per partition (4KB descriptor lines instead of 1KB)
    xT_d = nc.dram_tensor("xq", [NQ * 4 * 128, 4, QT], bf16,
                          kind="ExternalInput")
    wq_d = nc.dram_tensor("wq", [4 * 128, 4, HG], bf16, kind="ExternalInput")
    wk_d = nc.dram_tensor("wk", [4 * 128, 4, HG], bf16, kind="ExternalInput")
    wv_d = nc.dram_tensor("wv", [4 * 128, 4, HG], bf16, kind="ExternalInput")
    wo_d = nc.dram_tensor("wo", [HG, T], bf16, kind="ExternalInput")
    bqk_d = nc.dram_tensor("bqk", [128, 2, NH], f32, kind="ExternalInput")
    # per-(head, 4c-jc) re-anchored ALiBi bias columns
    alibi_d = nc.dram_tensor("alibi", [128, NH * 16], f32, kind="ExternalInput")
    # per-(head, query-in-chunk) stabilization row, added rank-1
    mshift_d = nc.dram_tensor("mshift", [1, NH * QT], r32, kind="ExternalInput")
    # triangular causal mask for the 128x128 block at the tile diagonal
    masktri_d = nc.dram_tensor("masktri", [128, 128], bf16, kind="ExternalInput")
    ones_d = nc.dram_tensor("ones", [128, 128], r32, kind="ExternalInput")
    onesrow_d = nc.dram_tensor("onesrow", [1, 128], r32, kind="ExternalInput")
    identb_d = nc.dram_tensor("identb", [128, 128], bf16, kind="ExternalInput")
    tri01_d = nc.dram_tensor("tri01", [128, 128], bf16, kind="ExternalInput")
    # output tiles packed pairwise: [c*8 + oc//2] -> [128, 1024]
    yT_d = nc.dram_tensor("yT", [NQ * 8 * 128, 1024], bf16,
                          kind="ExternalOutput")

    with tile.TileContext(nc) as tc:
        with (
            tc.tile_pool(name="consts", bufs=1) as consts,
            tc.tile_pool(name="wslab", bufs=1) as wslab,
            tc.tile_pool(name="qkvp", bufs=1) as qkvp,
            tc.tile_pool(name="xp", bufs=3) as xp,
            tc.tile_pool(name="outfp", bufs=2) as outfp,
            tc.tile_pool(name="pp", bufs=4) as pp,
            tc.tile_pool(name="rcpp", bufs=2) as rcpp,
            tc.tile_pool(name="paccp", bufs=2) as paccp,
            tc.tile_pool(name="ysb", bufs=6) as ysb,
            tc.tile_pool(name="ps_p1", bufs=2, space="PSUM") as ps_p1,
            tc.tile_pool(name="ps_s", bufs=3, space="PSUM") as ps_s,
            tc.tile_pool(name="ps_o", bufs=2, space="PSUM") as ps_o,
            tc.tile_pool(name="ps_d", bufs=1, space="PSUM") as ps_d,
        ):
            def xq_src(q, g):
                i = (q * 4 + g) * 128
                return xT_d.ap()[i:i + 128]

            def w_src(w_d, g):
                return w_d.ap()[g * 128:(g + 1) * 128]

            def body():
                # ---- startup loads: one list in need-order, issued
                # round-robin across the 3 DMA-capable queues so each
                # tensor lands roughly when its first consumer runs ----
                x_t = [[None] * 4 for _ in range(NQ)]
                wq_p, wk_p, wv_p = [], [], []
                bqk_sb = consts.tile([128, 2, NH], f32, name="bqk_sb")
                alibi_sb = consts.tile([128, NH * 16], f32, name="alibi_sb")
                masktri_sb = consts.tile([128, 128], bf16, name="masktri_sb")
                ones_sb = consts.tile([128, 128], r32, name="ones_sb")
                onesr_sb = consts.tile([1, 128], r32, name="onesr_sb")
                identb_sb = consts.tile([128, 128], bf16, name="identb_sb")
                tri01_sb = consts.tile([128, 128], bf16, name="tri01_sb")
                msh_sb = consts.tile([1, NH * QT], r32, name="msh_sb")
                wo_sb = wslab.tile([128, NH, T], bf16, name="wo_sb")
                # part g=0 of x chunk 0 and of wq lives in two half
                # tiles so the first matmuls wait on a ~0.7us DMA pair,
                # not the whole 512KB parts (readers wait on all writers
                # of a tile, so sub-tile DMA splits don't help)
                x0h = [wslab.tile([128, 2, QT], bf16, name=f"x0h{i}")
                       for i in range(2)]
                wq0h = [wslab.tile([128, 2, HG], bf16, name=f"wq0h{i}")
                        for i in range(2)]
                for g in range(4):
                    if g > 0:
                        x_t[0][g] = xp.tile([128, 4, QT], bf16,
                                            name=f"x_p{g}", tag=f"x_p{g}")
                    x_t[1][g] = xp.tile([128, 4, QT], bf16, name=f"x_p{g}",
                                        tag=f"x_p{g}")
                    wq_p.append(None if g == 0 else
                                wslab.tile([128, 4, HG], bf16,
                                           name=f"wq_p{g}"))
                    wk_p.append(wslab.tile([128, 4, HG], bf16,
                                           name=f"wk_p{g}"))
                    wv_p.append(wslab.tile([128, 4, HG], bf16,
                                           name=f"wv_p{g}"))

                def x0_sl(e, cols=slice(None)):
                    if e < 4:
                        return x0h[e // 2][:, e % 2, cols]
                    return x_t[0][e // 4][:, e % 4, cols]

                def wq_sl(e, cols):
                    if e < 4:
                        return wq0h[e // 2][:, e % 2, cols]
                    return wq_p[e // 4][:, e % 4, cols]

                wo_r = wo_d.ap().rearrange("(h p) o -> p h o", p=128)
                dmas = []  # (dst, src) in first-use order
                for g in range(4):
                    if g == 0:
                        dmas.append((x0h[0][:], xq_src(0, 0)[:, 0:2, :]))
                        dmas.append((wq0h[0][:],
                                     w_src(wq_d, 0)[:, 0:2, :]))
                        dmas.append((bqk_sb[:], bqk_d.ap()))
                        dmas.append((x0h[1][:], xq_src(0, 0)[:, 2:4, :]))
                        dmas.append((wq0h[1][:],
                                     w_src(wq_d, 0)[:, 2:4, :]))
                        continue
                    dmas.append((x_t[0][g][:], xq_src(0, g)))
                    dmas.append((wq_p[g][:], w_src(wq_d, g)))
                for g in range(4):
                    dmas.append((wk_p[g][:], w_src(wk_d, g)))
                dmas += [(alibi_sb[:], alibi_d.ap()),
                         (masktri_sb[:], masktri_d.ap()),
                         (ones_sb[:], ones_d.ap()),
                         (onesr_sb[:], onesrow_d.ap()),
                         (identb_sb[:], identb_d.ap()),
                         (tri01_sb[:], tri01_d.ap()),
                         (msh_sb[:], mshift_d.ap())]
                for g in range(4):
                    dmas.append((wv_p[g][:], w_src(wv_d, g)))
                # x chunk 1 prefetch (needed once proj(1) filler chains
                # start, well into attn(0)); wo only at the oproj(0)
                # fillers during attn(1)
                for g in range(4):
                    dmas.append((x_t[1][g][:], xq_src(1, g)))
                for h in range(NH):
                    dmas.append((wo_sb[:, h, :], wo_r[:, h, :]))
                qs = (nc.sync, nc.scalar, nc.gpsimd)
                for i, (dst, src) in enumerate(dmas):
                    qs[i % 3].dma_start(dst, src)

                qT_sb = [qkvp.tile([128, NH, QT], bf16, name=f"qT_sb{q}")
                         for q in range(NQ)]
                kT_sb = [qkvp.tile([128, NH, QT], bf16, name=f"kT_sb{q}")
                         for q in range(NQ)]
                v_sb = [qkvp.tile([128, 4, HG], bf16, name=f"v_sb{q}")
                        for q in range(NQ)]

                # ---- PE filler machinery: projection / output chains as
                # generators, one matmul emitted per pull ----
                def gen_qk_chain(c, pi, cc):
                    dst = (qT_sb, kT_sb)[pi]
                    csl = slice(cc * 128, (cc + 1) * 128)
                    ps = ps_p1.tile([128, 512], f32, name="p1acc",
                                    tag="p1acc")
                    for e in range(NE):
                        w_ap = (wq_sl(e, csl) if pi == 0 else
                                wk_p[e // 4][:, e % 4, csl])
                        nc.tensor.matmul(
                            ps[:], w_ap,
                            x_t[c][e // 4][:, e % 4, :],
                            start=(e == 0), stop=(e == NE - 1))
                        if e < NE - 1:
                            yield
                    nc.scalar.activation(
                        dst[c][:, cc, :], ps[:], Ident,
                        bias=bqk_sb[:, pi, cc:cc + 1])
                    yield

                def gen_v_chain(c, tb):
                    ps = ps_p1.tile([128, 512], f32, name="p1acc",
                                    tag="p1acc")
                    for e in range(NE):
                        nc.tensor.matmul(
                            ps[:],
                            x_t[c][e // 4][:, e % 4, tb * 128:(tb + 1) * 128],
                            wv_p[e // 4][:, e % 4, :],
                            start=(e == 0), stop=(e == NE - 1))
                        if e < NE - 1:
                            yield
                    nc.vector.tensor_copy(v_sb[c][:, tb, :], ps[:])
                    yield

                ys_pairs = {}

                def gen_oproj_chain(c, oc, outf_c):
                    if c == NQ - 1:
                        # final drain: attention is done, so the score /
                        # out PSUM pools and all copy engines are free —
                        # rotate across them so the 4-matmul chains never
                        # wait on a PSUM->SBUF copy
                        pool, ptag = [(ps_p1, "p1acc"), (ps_s, "s_ps"),
                                      (ps_o, "out_ps")][oc % 3]
                    else:
                        pool, ptag = ps_p1, "p1acc"
                    yp = pool.tile([128, 512], f32, name="y_ps", tag=ptag)
                    hord = (3, 2, 1, 0) if c == NQ - 1 else range(NH)
                    for hi, h in enumerate(hord):
                        nc.tensor.matmul(
                            yp[:],
                            wo_sb[:, h, oc * 128:(oc + 1) * 128],
                            outf_c[h][:],
                            start=(hi == 0), stop=(hi == 3))
                        if hi < NH - 1:
                            yield
                    # adjacent oc share one [128, 1024] SBUF tile so the
                    # store is a single 2KB-per-partition-line DMA
                    if oc % 2 == 0:
                        ys = ysb.tile([128, 1024], bf16, name="y_sb",
                                      tag="y_sb")
                        ys_pairs[c] = ys
                        nc.vector.tensor_copy(ys[:, 0:512], yp[:])
                        yield
                        return
                    ys = ys_pairs.pop(c)
                    nc.scalar.copy(ys[:, 512:1024], yp[:])
                    blk = (c * 8 + oc // 2) * 128
                    dst = yT_d.ap()[blk:blk + 128, :]
                    qeng = ([nc.gpsimd, nc.scalar, nc.sync][(oc // 2) % 3]
                            if c == NQ - 1 else
                            [nc.gpsimd, nc.scalar][(oc // 2) % 2])
                    qeng.dma_start(dst, ys[:])
                    yield

                filler = deque()

                def pull(n):
                    while n > 0 and filler:
                        try:
                            next(filler[0])
                            n -= 1
                        except StopIteration:
                            filler.popleft()

                def drain():
                    while filler:
                        try:
                            next(filler[0])
                        except StopIteration:
                            filler.popleft()

                def enqueue_proj(c):
                    for cc in range(4):
                        filler.append(gen_qk_chain(c, 0, cc))
                    for cc in range(4):
                        filler.append(gen_qk_chain(c, 1, cc))
                    for tb in range(4):
                        filler.append(gen_v_chain(c, tb))

                # ============ proj(0): standalone ============
                # Part-major with 4 parallel PSUM accumulators (the
                # score/out/den pools are all free at startup), so the PE
                # consumes each x/w part as its DMA lands instead of
                # serializing whole chains behind the 2-buffer p1 pool.
                def proj0_phase(accs, stat_of, mov_of, emit_copy):
                    for e in range(NE):
                        for j in range(4):
                            nc.tensor.matmul(
                                accs[j][:], stat_of(e, j), mov_of(e, j),
                                start=(e == 0), stop=(e == NE - 1))
                    for j in range(4):
                        emit_copy(j, accs[j])

                qk_accs = lambda: [
                    ps_p1.tile([128, 512], f32, name="p1acc", tag="p1acc"),
                    ps_p1.tile([128, 512], f32, name="p1acc", tag="p1acc"),
                    ps_s.tile([128, 512], f32, name="s_ps", tag="s_ps"),
                    ps_s.tile([128, 512], f32, name="s_ps", tag="s_ps"),
                ]
                proj0_phase(
                    qk_accs(),
                    lambda e, cc: wq_sl(e, slice(cc * 128, (cc + 1) * 128)),
                    lambda e, cc: x0_sl(e),
                    lambda cc, ps: nc.scalar.activation(
                        qT_sb[0][:, cc, :], ps[:], Ident,
                        bias=bqk_sb[:, 0, cc:cc + 1]))
                proj0_phase(
                    [ps_o.tile([128, 512], f32, name="out_ps", tag="out_ps"),
                     ps_o.tile([128, 512], f32, name="out_ps", tag="out_ps"),
                     ps_s.tile([128, 512], f32, name="s_ps", tag="s_ps"),
                     ps_d.tile([128, 512], f32, name="den_ps", tag="den_ps")],
                    lambda e, cc: wk_p[e // 4][:, e % 4,
                                               cc * 128:(cc + 1) * 128],
                    lambda e, cc: x0_sl(e),
                    lambda cc, ps: nc.scalar.activation(
                        kT_sb[0][:, cc, :], ps[:], Ident,
                        bias=bqk_sb[:, 1, cc:cc + 1]))
                proj0_phase(
                    qk_accs(),
                    lambda e, tb: x0_sl(e, slice(tb * 128, (tb + 1) * 128)),
                    lambda e, tb: wv_p[e // 4][:, e % 4, :],
                    lambda tb, ps: nc.vector.tensor_copy(
                        v_sb[0][:, tb, :], ps[:]))

                for c in range(NQ):
                    # x for chunk c+2 loads during attn(c) (x1 was loaded
                    # at startup); needed by proj(c+2) fillers in attn(c+1)
                    if 2 <= c + 2 < NQ:
                        for g in range(4):
                            x_t[c + 2][g] = xp.tile(
                                [128, 4, QT], bf16,
                                name=f"x_p{g}", tag=f"x_p{g}")
                            nc.sync.dma_start(
                                x_t[c + 2][g][:], xq_src(c + 2, g))

                    # proj(c+1) chains become filler for attn(c), behind
                    # any oproj(c-1) chains still queued
                    if c + 1 < NQ:
                        enqueue_proj(c + 1)

                    # ================ attn(c) ================
                    # per-head outf tiles keep oproj from waiting on the
                    # later heads' normalization tails
                    outf_c = [outfp.tile([128, 512], bf16,
                                         name=f"outf_h{h}", tag=f"outf_h{h}")
                              for h in range(NH)]
                    # last chunk: biggest head first so oproj(NQ-1) chains
                    # (accumulated in the same order) start while the
                    # small heads' softmax tails are still normalizing
                    horder = (3, 2, 1, 0) if c == NQ - 1 else range(NH)
                    for h in horder:
                        msh = needs_msh(c, h)
                        kept = list(kept_range(c, h))
                        first, last = kept[0], kept[-1]

                        s_tiles = {}
                        p_tiles = {}

                        def qlo_of(jc):
                            # queries below 128*(jc-4c) see no valid key
                            # in diag tile jc: skip that region entirely
                            return max(0, (jc - 4 * c) * 128)

                        def emit_score(jc):
                            # causal mask (one 128x128 triangle) and the
                            # rank-1 stabilization shift accumulate onto
                            # the score PSUM on the PE itself.
                            # The start matmul is always full-width so the
                            # PSUM bank has a uniform accumulation state.
                            s = ps_s.tile([128, 512], f32, name="s_ps",
                                          tag="s_ps")
                            diag = jc >= 4 * c
                            dlo = qlo_of(jc)
                            slo = 0 if sim_safe else dlo
                            mask_mm = diag and msh
                            nc.tensor.matmul(
                                s[:, slo:],
                                kT_sb[jc // 4][:, h,
                                               (jc % 4) * 128:
                                               (jc % 4 + 1) * 128],
                                qT_sb[c][:, h, slo:],
                                start=True, stop=not (mask_mm or msh))
                            if mask_mm:
                                nc.tensor.matmul(
                                    s[:, dlo:dlo + 128],
                                    identb_sb[:], masktri_sb[:],
                                    start=False, stop=not msh)
                            if msh:
                                mlo = min(dlo, 256)
                                nc.tensor.matmul(
                                    s[:, mlo:],
                                    onesr_sb[:],
                                    msh_sb[0:1, h * QT + mlo:
                                           (h + 1) * QT],
                                    start=False, stop=True)
                            s_tiles[jc] = s

                        def emit_exp(jc):
                            qlo = qlo_of(jc)
                            p = pp.tile([128, 512], bf16, name="p_sb",
                                        tag="p_sb")
                            nc.scalar.activation(
                                p[:, qlo:], s_tiles.pop(jc)[:, qlo:], Exp,
                                bias=alibi_sb[:, h * 16 + 4 * c - jc + 3:
                                              h * 16 + 4 * c - jc + 4])
                            if jc >= 4 * c and not msh:
                                # causal mask applied on p just ahead of
                                # the pacc add in the DVE FIFO; saves the
                                # PSUM mask matmul on the PE
                                nc.vector.tensor_tensor(
                                    p[:, qlo:qlo + 128],
                                    p[:, qlo:qlo + 128], tri01_sb[:],
                                    mybir.AluOpType.mult)
                            p_tiles[jc] = p

                        outp = ps_o.tile([128, 512], f32, name="out_ps",
                                         tag="out_ps")
                        # p accumulates on DVE (SBUF only); den is one
                        # matmul per head instead of one per tile
                        pacc = paccp.tile([128, 512], r32, name="pacc_sb",
                                          tag="pacc_sb")

                        def emit_consume(jc):
                            p = p_tiles.pop(jc)
                            qlo = qlo_of(jc)
                            if jc == first:
                                nc.vector.tensor_copy(pacc[:], p[:])
                            else:
                                nc.vector.tensor_tensor(
                                    pacc[:, qlo:], pacc[:, qlo:],
                                    p[:, qlo:], AddOp)
                            nc.tensor.matmul(
                                outp[:, qlo:],
                                v_sb[jc // 4][:, jc % 4,
                                              h * 128:(h + 1) * 128],
                                p[:, qlo:],
                                start=(jc == first), stop=(jc == last))

                        for i in range(min(LOOK, len(kept))):
                            emit_score(kept[i])
                        for i, jc in enumerate(kept):
                            if i + LOOK < len(kept):
                                emit_score(kept[i + LOOK])
                            pull(2)
                            emit_exp(jc)
                            emit_consume(jc)

                        # filler before den hides the DVE pacc tail
                        pull(3)
                        den = ps_d.tile([128, 512], f32, name="den_ps",
                                        tag="den_ps")
                        nc.tensor.matmul(den[:], ones_sb[:], pacc[:],
                                         start=True, stop=True)
                        rcp = rcpp.tile([128, 512], f32, name="rcp",
                                        tag="rcp")
                        with nc.allow_low_precision(
                                reason="elementwise reciprocal"):
                            nc.vector.reciprocal(rcp[:], den[:])
                        nc.vector.tensor_mul(
                            outf_c[h][:], outp[:], rcp[:])
                        pull(2)

                    # finish oproj(c-1) + proj(c+1) chains before oproj(c)
                    drain()

                    # oproj(c) chains: filler for attn(c+1) (or drained
                    # at the end for the last chunk)
                    for oc in range(16):
                        filler.append(gen_oproj_chain(c, oc, outf_c))

                drain()

            if reps == 1:
                body()
            else:
                # unroll the rep loop: each For_i iteration carries an
                # all-engine barrier, so amortize it over several bodies
                u = 1
                for cand in (4, 3, 2):
                    if reps % cand == 0:
                        u = cand
                        break
                with tc.For_i(0, reps // u, 1):
                    for _ in range(u):
                        body()

    nc.compile()
    return nc


def get_slopes():
    start = 2 ** (-2 ** (-(math.log2(16) - 3)))
    return np.array([start * start ** i for i in range(16)], np.float32)


def make_host_inputs(x, Wq, bq, Wk, bk, Wv, bv, Wo, bo):
    """Shard full inputs into 8 per-core input maps."""
    nbf16 = mybir.dt.np(bf16)
    x = np.asarray(x, np.float32)
    Wq = np.asarray(Wq, np.float32); bq = np.asarray(bq, np.float32)
    Wk = np.asarray(Wk, np.float32); bk = np.asarray(bk, np.float32)
    Wv = np.asarray(Wv, np.float32)
    Wo = np.asarray(Wo, np.float32)

    slopes = get_slopes()
    sc = np.float32(1.0 / math.sqrt(HD))
    jl = np.arange(128, dtype=np.float32)
    il = np.arange(QT, dtype=np.float32)

    masktri = np.where(jl[:, None] > np.arange(128)[None, :],
                       np.float32(-1e10), np.float32(0.0))
    tri01 = np.where(jl[:, None] > np.arange(128)[None, :],
                     np.float32(0.0), np.float32(1.0))
    ones128 = np.ones((128, 128), np.float32)
    onesrow = np.ones((1, 128), np.float32)
    identb = np.eye(128, dtype=np.float32)

    in_maps = []
    for core in range(8):
        b, s = core // 4, core % 4
        heads = [s, s + 4, s + 8, s + 12]
        cols = np.concatenate(
            [np.arange(h * HD, (h + 1) * HD) for h in heads])
        # bias column for tile (c, jc): anchored at the last query of
        # chunk c:  b[jl] = -slope * (128*(4c - jc) + 511 - jl)
        alibi = np.empty((128, NH * 16), np.float32)
        mshift = np.empty((1, NH * QT), np.float32)
        for hh, h in enumerate(heads):
            for dd in range(16):
                kk = dd - 3  # 4c - jc
                alibi[:, hh * 16 + dd] = -slopes[h] * (
                    128.0 * kk + 511.0 - jl)
            mshift[0, hh * QT:(hh + 1) * QT] = slopes[h] * (511.0 - il)
        bqk = np.zeros((128, 2, NH), np.float32)
        bqk[:, 0, :] = (bq[cols] * sc).reshape(NH, HD).T
        bqk[:, 1, :] = bk[cols].reshape(NH, HD).T
        def pack_x(xb):
            # [emb, t] -> tile-packed [(q*4+g)*128+p, c, t]
            a = xb.reshape(4, 4, 128, 4, 512)          # [g, c, p, q, t]
            return np.ascontiguousarray(
                a.transpose(3, 0, 2, 1, 4).reshape(16 * 128, 4, 512))

        def pack_w(w):
            # [emb, m] -> [(g*128+p), c, m]
            a = w.reshape(4, 4, 128, HG)               # [g, c, p, m]
            return np.ascontiguousarray(
                a.transpose(0, 2, 1, 3).reshape(4 * 128, 4, HG))

        in_maps.append({
            "xq": pack_x(x[b].T).astype(nbf16),
            "wq": pack_w(Wq[:, cols] * sc).astype(nbf16),
            "wk": pack_w(Wk[:, cols]).astype(nbf16),
            "wv": pack_w(Wv[:, cols]).astype(nbf16),
            "wo": np.ascontiguousarray(Wo[cols, :]).astype(nbf16),
            "bqk": bqk,
            "alibi": alibi,
            "mshift": mshift,
            "masktri": masktri.astype(nbf16),
            "ones": ones128,
            "onesrow": onesrow,
            "identb": identb.astype(nbf16),
            "tri01": tri01.astype(nbf16),
        })
    return in_maps


def assemble_output(results, Wv_bias=None, bo=None, Wo=None):
    """results: list of 8 per-core dicts with 'yT' (bf16 partials).

    v-bias folds out of attention exactly: out_h = attn(v'_h) + bv_h,
    so y = sum_h out_h Wo_h = y' + bv @ Wo. Added here with bo.
    Positional-compat: assemble_output(results, bo) treats bv as zero.
    """
    if bo is None:
        Wv_bias, bo = None, Wv_bias
    bo = np.asarray(bo, np.float32)
    if Wv_bias is not None and Wo is not None and np.any(Wv_bias):
        extra = np.asarray(Wv_bias, np.float32) @ np.asarray(Wo, np.float32) + bo
    else:
        extra = bo  # (2048,)
    out = np.empty((2, T, EMB), np.float32)
    for b in range(2):
        acc = np.asarray(results[b * 4 + 0]["yT"], np.float32)
        for s in range(1, 4):
            acc += np.asarray(results[b * 4 + s]["yT"], np.float32)
        # un-pack the pairwise tile layout [c*8+j, 128, 2, 512] back to
        # [emb_out, tok]
        acc = acc.reshape(4, 8, 128, 2, 512).transpose(
            1, 3, 2, 0, 4).reshape(T, T)
        out[b] = acc.T + extra
    return out


class SpmdRunner:
    def __init__(self, nc, n_cores: int):
        install_neuronx_cc_hook()
        self.nc = nc
        self.n_cores = n_cores
        assert nc.dbg_addr is None or not nc.dbg_callbacks
        partition_name = (
            nc.partition_id_tensor.name if nc.partition_id_tensor else None
        )
        in_names, out_names, out_avals = [], [], []
        for alloc in nc.m.functions[0].allocations:
            if not isinstance(alloc, mybir.MemoryLocationSet):
                continue
            name = alloc.memorylocations[0].name
            if alloc.kind == "ExternalInput":
                if name != partition_name:
                    in_names.append(name)
            elif alloc.kind == "ExternalOutput":
                shape = tuple(alloc.tensor_shape)
                dtype = mybir.dt.np(alloc.dtype)
                out_names.append(name)
                out_avals.append(jax.core.ShapedArray(shape, dtype))
        self.in_names = list(in_names)
        self.out_names = out_names
        self.out_avals = out_avals
        n_params = len(self.in_names)
        all_in_names = list(in_names) + list(out_names)
        if partition_name is not None:
            all_in_names.append(partition_name)
        self.partition_name = partition_name

        def _body(*args):
            operands = list(args)
            if partition_name is not None:
                operands.append(bass2jax.partition_id_tensor())
            outs = _bass_exec_p.bind(
                *operands,
                out_avals=tuple(out_avals),
                in_names=tuple(all_in_names),
                out_names=tuple(out_names),
                lowering_input_output_aliases=(),
                sim_require_finite=True,
                sim_require_nnan=True,
                nc=nc,
            )
            return tuple(outs)

        devices = jax.devices()[:n_cores]
        assert len(devices) == n_cores
        self.mesh = Mesh(np.asarray(devices), ("core",))
        n_outs = len(out_names)
        in_specs = (PartitionSpec("core"),) * (n_params + n_outs)
        out_specs = (PartitionSpec("core"),) * n_outs
        self.fn = jax.jit(
            shard_map(_body, mesh=self.mesh, in_specs=in_specs,
                      out_specs=out_specs, check_rep=False),
            keep_unused=True,
        )
        self.dev_args = None

    def set_inputs(self, in_maps: list[dict]):
        """device_put concatenated per-core inputs + zero output buffers."""
        n = self.n_cores
        assert len(in_maps) == n
        concat_in = [
            np.concatenate([np.asarray(in_maps[c][name]) for c in range(n)], axis=0)
            for name in self.in_names
        ]
        concat_zeros = [
            np.zeros((n * a.shape[0], *a.shape[1:]), a.dtype) for a in self.out_avals
        ]
        sharding = jax.sharding.NamedSharding(self.mesh, PartitionSpec("core"))
        self.dev_args = [jax.device_put(a, sharding) for a in concat_in + concat_zeros]

    def run(self):
        outs = self.fn(*self.dev_args)
        jax.block_until_ready(outs)
        return outs

    def results(self, outs) -> list[dict]:
        n = self.n_cores
        return [
            {
                name: np.asarray(outs[i]).reshape(n, *self.out_avals[i].shape)[c]
                for i, name in enumerate(self.out_names)
            }
            for c in range(n)
        ]

    def time_execs(self, iters: int = 10, warmup: int = 2):
        for _ in range(warmup):
            self.run()
        t0 = time.perf_counter()
        for _ in range(iters):
            outs = self.fn(*self.dev_args)
        jax.block_until_ready(outs)
        t1 = time.perf_counter()
        return (t1 - t0) / iters


_RUNNER = None


def _get_runner():
    global _RUNNER
    if _RUNNER is None:
        nc = build_program(reps=1)
        _RUNNER = SpmdRunner(nc, 8)
    return _RUNNER


def kernel(x, Wq, bq, Wk, bk, Wv, bv, Wo, bo):
    r = _get_runner()
    in_maps = make_host_inputs(x, Wq, bq, Wk, bk, Wv, bv, Wo, bo)
    r.set_inputs(in_maps)
    outs = r.run()
    res = r.results(outs)
    return assemble_output(res, bv, bo, Wo)


# revision 29
# speedup vs baseline: 1.0259x; 1.0259x over previous
"""Trainium2 Bass kernel for nn_CausalAttention_84018150244353.

kernel(**inputs) takes the FULL unsharded inputs (as in reference
setup_inputs) and returns the full (2, 2048, 2048) float32 output.

Sharding: 8 NeuronCores = 2 batches x 4 head-slots. Heads are grouped
into 4 work classes by ALiBi slope (large slopes attend only a short
window, so distant key chunks are dropped); each core gets one head of
each class so all cores run the identical program with balanced work:
  slot 0 (heads 0-3):   5 key chunks per query chunk
  slot 1 (heads 4-7):   5 key chunks
  slot 2 (heads 8-11):  8 key chunks
  slot 3 (heads 12-15): full causal
Core (b, s) handles batch b and heads {s, s+4, s+8, s+12}.

Per-core program, software-pipelined so the PE never waits on the
Scalar engine's exp chain:
  proj(0) runs standalone; thereafter the q/k/v projection chains for
  chunk c+1 and the output-projection chains for chunk c-1 are pulled
  as PE "filler" work between attention tiles of chunk c (the exp on
  Scalar is the per-tile rate limiter; two 512-col filler matmuls per
  tile cover the deficit). The exp-sum accumulator (pacc) and the
  PSUM->SBUF copies run on the Vector engine (DVE).
  Host sums the 4 head-slot partials, adds bo + bv @ Wo.
"""
import math
import os
import sys
import time
from collections import deque

sys.path.insert(0, "/opt/trn_rl_repo")

import numpy as np
import jax

jax.config.update("jax_compilation_cache_dir",
                  os.environ.get("JAX_NEFF_CACHE", "/tmp/jax_neff_cache"))
jax.config.update("jax_persistent_cache_min_compile_time_secs", 0.0)
jax.config.update("jax_persistent_cache_min_entry_size_bytes", -1)

from jax.sharding import Mesh, PartitionSpec
from jax.experimental.shard_map import shard_map

import concourse.bass as bass
import concourse.mybir as mybir
import concourse.tile as tile
from concourse import bacc
from concourse import bass2jax
from concourse.bass2jax import _bass_exec_p, install_neuronx_cc_hook

f32 = mybir.dt.float32
r32 = mybir.dt.float32r
bf16 = mybir.dt.bfloat16
Exp = mybir.ActivationFunctionType.Exp
Ident = mybir.ActivationFunctionType.Identity
AddOp = mybir.AluOpType.add

T = 2048
EMB = 2048
HG = 512          # columns per core (4 heads x 128)
HD = 128
NH = 4            # heads per core
NQ = 4            # T quarters
QT = T // NQ      # 512
NE = EMB // 128   # 16 contraction chunks
NJ = T // 128     # 16 key chunks
KS = (5, 5, 8, 16)    # kept key chunks per head slot (ALiBi cutoff)
LOOK = 3


def kept_range(c: int, s: int):
    return range(max(0, 4 * c + 4 - KS[s]), 4 * c + 4)


# Max ALiBi slope of the heads a slot can hold (slot s holds heads
# {s, s+4, s+8, s+12}; classes are h//4).
_SLOT_MAX_SLOPE = (2.0 ** -0.5, 2.0 ** -2.5, 2.0 ** -4.5, 2.0 ** -6.5)


def needs_msh(c: int, s: int) -> bool:
    """Whether exp() needs the rank-1 per-query stabilization shift.
    With the bias re-anchored at the last query of chunk c, the exponent
    floor is -slope * max-key-depth; fp32 exp stays in normal range
    (given |s| <~ 8) as long as that is above ~-70."""
    depth = min(512 * c + 511, 511 + 128 * (KS[s] - 4))
    return _SLOT_MAX_SLOPE[s] * depth > 70.0


def build_program(reps: int = 1, sim_safe: bool = False):
    """sim_safe=True keeps start-matmuls full-width so exec-mode CoreSim's
    PSUM pending-zero bookkeeping holds (hardware is fine either way)."""
    nc = bacc.Bacc("TRN2", target_bir_lowering=False, debug=False,
                   enable_asserts=False, num_devices=8)

    # x and w are tile-packed host-side so every load is fully
    # contiguous per partition (4KB descriptor lines instead of 1KB)
    xT_d = nc.dram_tensor("xq", [NQ * 4 * 128, 4, QT], bf16,
                          kind="ExternalInput")
    wq_d = nc.dram_tensor("wq", [4 * 128, 4, HG], bf16, kind="ExternalInput")
    wk_d = nc.dram_tensor("wk", [4 * 128, 4, HG], bf16, kind="ExternalInput")
    wv_d = nc.dram_tensor("wv", [4 * 128, 4, HG], bf16, kind="ExternalInput")
    wo_d = nc.dram_tensor("wo", [HG, T], bf16, kind="ExternalInput")
    bqk_d = nc.dram_tensor("bqk", [128, 2, NH], f32, kind="ExternalInput")
    # per-(head, 4c-jc) re-anchored ALiBi bias columns
    alibi_d = nc.dram_tensor("alibi", [128, NH * 16], f32, kind="ExternalInput")
    # per-(head, query-in-chunk) stabilization row, added rank-1
    mshift_d = nc.dram_tensor("mshift", [1, NH * QT], r32, kind="ExternalInput")
    # triangular causal mask for the 128x128 block at the tile diagonal
    masktri_d = nc.dram_tensor("masktri", [128, 128], bf16, kind="ExternalInput")
    ones_d = nc.dram_tensor("ones", [128, 128], r32, kind="ExternalInput")
    onesrow_d = nc.dram_tensor("onesrow", [1, 128], r32, kind="ExternalInput")
    identb_d = nc.dram_tensor("identb", [128, 128], bf16, kind="ExternalInput")
    tri01_d = nc.dram_tensor("tri01", [128, 128], bf16, kind="ExternalInput")
    # output tiles packed pairwise: [c*8 + oc//2] -> [128, 1024]
    yT_d = nc.dram_tensor("yT", [NQ * 8 * 128, 1024], bf16,
                          kind="ExternalOutput")

    with tile.TileContext(nc) as tc:
        with (
            tc.tile_pool(name="consts", bufs=1) as consts,
            tc.tile_pool(name="wslab", bufs=1) as wslab,
            tc.tile_pool(name="qkvp", bufs=1) as qkvp,
            tc.tile_pool(name="xp", bufs=3) as xp,
            tc.tile_pool(name="outfp", bufs=2) as outfp,
            tc.tile_pool(name="pp", bufs=4) as pp,
            tc.tile_pool(name="rcpp", bufs=2) as rcpp,
            tc.tile_pool(name="paccp", bufs=2) as paccp,
            tc.tile_pool(name="ysb", bufs=6) as ysb,
            tc.tile_pool(name="ps_p1", bufs=2, space="PSUM") as ps_p1,
            tc.tile_pool(name="ps_s", bufs=3, space="PSUM") as ps_s,
            tc.tile_pool(name="ps_o", bufs=2, space="PSUM") as ps_o,
            tc.tile_pool(name="ps_d", bufs=1, space="PSUM") as ps_d,
        ):
            def xq_src(q, g):
                i = (q * 4 + g) * 128
                return xT_d.ap()[i:i + 128]

            def w_src(w_d, g):
                return w_d.ap()[g * 128:(g + 1) * 128]

            def body():
                # ---- startup loads: one list in need-order, issued
                # round-robin across the 3 DMA-capable queues so each
                # tensor lands roughly when its first consumer runs ----
                x_t = [[None] * 4 for _ in range(NQ)]
                wq_p, wk_p, wv_p = [], [], []
                bqk_sb = consts.tile([128, 2, NH], f32, name="bqk_sb")
                alibi_sb = consts.tile([128, NH * 16], f32, name="alibi_sb")
                masktri_sb = consts.tile([128, 128], bf16, name="masktri_sb")
                ones_sb = consts.tile([128, 128], r32, name="ones_sb")
                onesr_sb = consts.tile([1, 128], r32, name="onesr_sb")
                identb_sb = consts.tile([128, 128], bf16, name="identb_sb")
                tri01_sb = consts.tile([128, 128], bf16, name="tri01_sb")
                msh_sb = consts.tile([1, NH * QT], r32, name="msh_sb")
                wo_sb = wslab.tile([128, NH, T], bf16, name="wo_sb")
                # part g=0 of x chunk 0 and of wq lives in two half
                # tiles so the first matmuls wait on a ~0.7us DMA pair,
                # not the whole 512KB parts (readers wait on all writers
                # of a tile, so sub-tile DMA splits don't help)
                x0h = [wslab.tile([128, 1, QT], bf16, name=f"x0h{i}")
                       for i in range(4)]
                wq0h = [wslab.tile([128, 1, HG], bf16, name=f"wq0h{i}")
                        for i in range(4)]
                for g in range(4):
                    if g > 0:
                        x_t[0][g] = xp.tile([128, 4, QT], bf16,
                                            name=f"x_p{g}", tag=f"x_p{g}")
                    x_t[1][g] = xp.tile([128, 4, QT], bf16, name=f"x_p{g}",
                                        tag=f"x_p{g}")
                    wq_p.append(None if g == 0 else
                                wslab.tile([128, 4, HG], bf16,
                                           name=f"wq_p{g}"))
                    wk_p.append(wslab.tile([128, 4, HG], bf16,
                                           name=f"wk_p{g}"))
                    wv_p.append(wslab.tile([128, 4, HG], bf16,
                                           name=f"wv_p{g}"))

                def x0_sl(e, cols=slice(None)):
                    if e < 4:
                        return x0h[e][:, 0, cols]
                    return x_t[0][e // 4][:, e % 4, cols]

                def wq_sl(e, cols):
                    if e < 4:
                        return wq0h[e][:, 0, cols]
                    return wq_p[e // 4][:, e % 4, cols]

                wo_r = wo_d.ap().rearrange("(h p) o -> p h o", p=128)
                dmas = []  # (dst, src) in first-use order
                for g in range(4):
                    if g == 0:
                        for e in range(4):
                            dmas.append((x0h[e][:],
                                         xq_src(0, 0)[:, e:e + 1, :]))
                            dmas.append((wq0h[e][:],
                                         w_src(wq_d, 0)[:, e:e + 1, :]))
                            if e == 0:
                                dmas.append((bqk_sb[:], bqk_d.ap()))
                        continue
                    dmas.append((x_t[0][g][:], xq_src(0, g)))
                    dmas.append((wq_p[g][:], w_src(wq_d, g)))
                for g in range(4):
                    dmas.append((wk_p[g][:], w_src(wk_d, g)))
                dmas += [(alibi_sb[:], alibi_d.ap()),
                         (masktri_sb[:], masktri_d.ap()),
                         (ones_sb[:], ones_d.ap()),
                         (onesr_sb[:], onesrow_d.ap()),
                         (identb_sb[:], identb_d.ap()),
                         (tri01_sb[:], tri01_d.ap()),
                         (msh_sb[:], mshift_d.ap())]
                for g in range(4):
                    dmas.append((wv_p[g][:], w_src(wv_d, g)))
                # x chunk 1 prefetch (needed once proj(1) filler chains
                # start, well into attn(0)); wo only at the oproj(0)
                # fillers during attn(1)
                for g in range(4):
                    dmas.append((x_t[1][g][:], xq_src(1, g)))
                for h in range(NH):
                    dmas.append((wo_sb[:, h, :], wo_r[:, h, :]))
                qs = (nc.sync, nc.scalar, nc.gpsimd)
                for i, (dst, src) in enumerate(dmas):
                    qs[i % 3].dma_start(dst, src)

                qT_sb = [qkvp.tile([128, NH, QT], bf16, name=f"qT_sb{q}")
                         for q in range(NQ)]
                kT_sb = [qkvp.tile([128, NH, QT], bf16, name=f"kT_sb{q}")
                         for q in range(NQ)]
                v_sb = [qkvp.tile([128, 4, HG], bf16, name=f"v_sb{q}")
                        for q in range(NQ)]

                # ---- PE filler machinery: projection / output chains as
                # generators, one matmul emitted per pull ----
                def gen_qk_chain(c, pi, cc):
                    dst = (qT_sb, kT_sb)[pi]
                    csl = slice(cc * 128, (cc + 1) * 128)
                    ps = ps_p1.tile([128, 512], f32, name="p1acc",
                                    tag="p1acc")
                    for e in range(NE):
                        w_ap = (wq_sl(e, csl) if pi == 0 else
                                wk_p[e // 4][:, e % 4, csl])
                        nc.tensor.matmul(
                            ps[:], w_ap,
                            x_t[c][e // 4][:, e % 4, :],
                            start=(e == 0), stop=(e == NE - 1))
                        if e < NE - 1:
                            yield
                    nc.scalar.activation(
                        dst[c][:, cc, :], ps[:], Ident,
                        bias=bqk_sb[:, pi, cc:cc + 1])
                    yield

                def gen_v_chain(c, tb):
                    ps = ps_p1.tile([128, 512], f32, name="p1acc",
                                    tag="p1acc")
                    for e in range(NE):
                        nc.tensor.matmul(
                            ps[:],
                            x_t[c][e // 4][:, e % 4, tb * 128:(tb + 1) * 128],
                            wv_p[e // 4][:, e % 4, :],
                            start=(e == 0), stop=(e == NE - 1))
                        if e < NE - 1:
                            yield
                    nc.vector.tensor_copy(v_sb[c][:, tb, :], ps[:])
                    yield

                ys_pairs = {}

                def gen_oproj_chain(c, oc, outf_c):
                    if c == NQ - 1:
                        # final drain: attention is done, so the score /
                        # out PSUM pools and all copy engines are free —
                        # rotate across them so the 4-matmul chains never
                        # wait on a PSUM->SBUF copy
                        pool, ptag = [(ps_p1, "p1acc"), (ps_s, "s_ps"),
                                      (ps_o, "out_ps")][oc % 3]
                    else:
                        pool, ptag = ps_p1, "p1acc"
                    yp = pool.tile([128, 512], f32, name="y_ps", tag=ptag)
                    hord = (3, 2, 1, 0) if c == NQ - 1 else range(NH)
                    for hi, h in enumerate(hord):
                        nc.tensor.matmul(
                            yp[:],
                            wo_sb[:, h, oc * 128:(oc + 1) * 128],
                            outf_c[h][:],
                            start=(hi == 0), stop=(hi == 3))
                        if hi < NH - 1:
                            yield
                    # adjacent oc share one [128, 1024] SBUF tile so the
                    # store is a single 2KB-per-partition-line DMA
                    if oc % 2 == 0:
                        ys = ysb.tile([128, 1024], bf16, name="y_sb",
                                      tag="y_sb")
                        ys_pairs[c] = ys
                        nc.vector.tensor_copy(ys[:, 0:512], yp[:])
                        yield
                        return
                    ys = ys_pairs.pop(c)
                    nc.scalar.copy(ys[:, 512:1024], yp[:])
                    blk = (c * 8 + oc // 2) * 128
                    dst = yT_d.ap()[blk:blk + 128, :]
                    qeng = ([nc.gpsimd, nc.scalar, nc.sync][(oc // 2) % 3]
                            if c == NQ - 1 else
                            [nc.gpsimd, nc.scalar][(oc // 2) % 2])
                    qeng.dma_start(dst, ys[:])
                    yield

                filler = deque()

                def pull(n):
                    while n > 0 and filler:
                        try:
                            next(filler[0])
                            n -= 1
                        except StopIteration:
                            filler.popleft()

                def drain():
                    while filler:
                        try:
                            next(filler[0])
                        except StopIteration:
                            filler.popleft()

                def enqueue_proj(c):
                    for cc in range(4):
                        filler.append(gen_qk_chain(c, 0, cc))
                    for cc in range(4):
                        filler.append(gen_qk_chain(c, 1, cc))
                    for tb in range(4):
                        filler.append(gen_v_chain(c, tb))

                # ============ proj(0): standalone ============
                # Part-major with 4 parallel PSUM accumulators (the
                # score/out/den pools are all free at startup), so the PE
                # consumes each x/w part as its DMA lands instead of
                # serializing whole chains behind the 2-buffer p1 pool.
                def proj0_phase(accs, stat_of, mov_of, emit_copy):
                    for e in range(NE):
                        for j in range(4):
                            nc.tensor.matmul(
                                accs[j][:], stat_of(e, j), mov_of(e, j),
                                start=(e == 0), stop=(e == NE - 1))
                    for j in range(4):
                        emit_copy(j, accs[j])

                qk_accs = lambda: [
                    ps_p1.tile([128, 512], f32, name="p1acc", tag="p1acc"),
                    ps_p1.tile([128, 512], f32, name="p1acc", tag="p1acc"),
                    ps_s.tile([128, 512], f32, name="s_ps", tag="s_ps"),
                    ps_s.tile([128, 512], f32, name="s_ps", tag="s_ps"),
                ]
                proj0_phase(
                    qk_accs(),
                    lambda e, cc: wq_sl(e, slice(cc * 128, (cc + 1) * 128)),
                    lambda e, cc: x0_sl(e),
                    lambda cc, ps: nc.scalar.activation(
                        qT_sb[0][:, cc, :], ps[:], Ident,
                        bias=bqk_sb[:, 0, cc:cc + 1]))
                proj0_phase(
                    [ps_o.tile([128, 512], f32, name="out_ps", tag="out_ps"),
                     ps_o.tile([128, 512], f32, name="out_ps", tag="out_ps"),
                     ps_s.tile([128, 512], f32, name="s_ps", tag="s_ps"),
                     ps_d.tile([128, 512], f32, name="den_ps", tag="den_ps")],
                    lambda e, cc: wk_p[e // 4][:, e % 4,
                                               cc * 128:(cc + 1) * 128],
                    lambda e, cc: x0_sl(e),
                    lambda cc, ps: nc.scalar.activation(
                        kT_sb[0][:, cc, :], ps[:], Ident,
                        bias=bqk_sb[:, 1, cc:cc + 1]))
                proj0_phase(
                    qk_accs(),
                    lambda e, tb: x0_sl(e, slice(tb * 128, (tb + 1) * 128)),
                    lambda e, tb: wv_p[e // 4][:, e % 4, :],
                    lambda tb, ps: nc.vector.tensor_copy(
                        v_sb[0][:, tb, :], ps[:]))

                for c in range(NQ):
                    # x for chunk c+2 loads during attn(c) (x1 was loaded
                    # at startup); needed by proj(c+2) fillers in attn(c+1)
                    if 2 <= c + 2 < NQ:
                        for g in range(4):
                            x_t[c + 2][g] = xp.tile(
                                [128, 4, QT], bf16,
                                name=f"x_p{g}", tag=f"x_p{g}")
                            nc.sync.dma_start(
                                x_t[c + 2][g][:], xq_src(c + 2, g))

                    # proj(c+1) chains become filler for attn(c), behind
                    # any oproj(c-1) chains still queued
                    if c + 1 < NQ:
                        enqueue_proj(c + 1)

                    # ================ attn(c) ================
                    # per-head outf tiles keep oproj from waiting on the
                    # later heads' normalization tails
                    outf_c = [outfp.tile([128, 512], bf16,
                                         name=f"outf_h{h}", tag=f"outf_h{h}")
                              for h in range(NH)]
                    # last chunk: biggest head first so oproj(NQ-1) chains
                    # (accumulated in the same order) start while the
                    # small heads' softmax tails are still normalizing
                    horder = (3, 2, 1, 0) if c == NQ - 1 else range(NH)
                    for h in horder:
                        msh = needs_msh(c, h)
                        kept = list(kept_range(c, h))
                        first, last = kept[0], kept[-1]

                        s_tiles = {}
                        p_tiles = {}

                        def qlo_of(jc):
                            # queries below 128*(jc-4c) see no valid key
                            # in diag tile jc: skip that region entirely
                            return max(0, (jc - 4 * c) * 128)

                        def emit_score(jc):
                            # causal mask (one 128x128 triangle) and the
                            # rank-1 stabilization shift accumulate onto
                            # the score PSUM on the PE itself.
                            # The start matmul is always full-width so the
                            # PSUM bank has a uniform accumulation state.
                            s = ps_s.tile([128, 512], f32, name="s_ps",
                                          tag="s_ps")
                            diag = jc >= 4 * c
                            dlo = qlo_of(jc)
                            slo = 0 if sim_safe else dlo
                            mask_mm = diag and msh
                            nc.tensor.matmul(
                                s[:, slo:],
                                kT_sb[jc // 4][:, h,
                                               (jc % 4) * 128:
                                               (jc % 4 + 1) * 128],
                                qT_sb[c][:, h, slo:],
                                start=True, stop=not (mask_mm or msh))
                            if mask_mm:
                                nc.tensor.matmul(
                                    s[:, dlo:dlo + 128],
                                    identb_sb[:], masktri_sb[:],
                                    start=False, stop=not msh)
                            if msh:
                                mlo = min(dlo, 256)
                                nc.tensor.matmul(
                                    s[:, mlo:],
                                    onesr_sb[:],
                                    msh_sb[0:1, h * QT + mlo:
                                           (h + 1) * QT],
                                    start=False, stop=True)
                            s_tiles[jc] = s

                        def emit_exp(jc):
                            qlo = qlo_of(jc)
                            p = pp.tile([128, 512], bf16, name="p_sb",
                                        tag="p_sb")
                            nc.scalar.activation(
                                p[:, qlo:], s_tiles.pop(jc)[:, qlo:], Exp,
                                bias=alibi_sb[:, h * 16 + 4 * c - jc + 3:
                                              h * 16 + 4 * c - jc + 4])
                            if jc >= 4 * c and not msh:
                                # causal mask applied on p just ahead of
                                # the pacc add in the DVE FIFO; saves the
                                # PSUM mask matmul on the PE
                                nc.vector.tensor_tensor(
                                    p[:, qlo:qlo + 128],
                                    p[:, qlo:qlo + 128], tri01_sb[:],
                                    mybir.AluOpType.mult)
                            p_tiles[jc] = p

                        outp = ps_o.tile([128, 512], f32, name="out_ps",
                                         tag="out_ps")
                        # p accumulates on DVE (SBUF only); den is one
                        # matmul per head instead of one per tile
                        pacc = paccp.tile([128, 512], r32, name="pacc_sb",
                                          tag="pacc_sb")

                        def emit_consume(jc):
                            p = p_tiles.pop(jc)
                            qlo = qlo_of(jc)
                            if jc == first:
                                nc.vector.tensor_copy(pacc[:], p[:])
                            else:
                                nc.vector.tensor_tensor(
                                    pacc[:, qlo:], pacc[:, qlo:],
                                    p[:, qlo:], AddOp)
                            nc.tensor.matmul(
                                outp[:, qlo:],
                                v_sb[jc // 4][:, jc % 4,
                                              h * 128:(h + 1) * 128],
                                p[:, qlo:],
                                start=(jc == first), stop=(jc == last))

                        for i in range(min(LOOK, len(kept))):
                            emit_score(kept[i])
                        for i, jc in enumerate(kept):
                            if i + LOOK < len(kept):
                                emit_score(kept[i + LOOK])
                            pull(2)
                            emit_exp(jc)
                            emit_consume(jc)

                        # filler before den hides the DVE pacc tail
                        pull(3)
                        den = ps_d.tile([128, 512], f32, name="den_ps",
                                        tag="den_ps")
                        nc.tensor.matmul(den[:], ones_sb[:], pacc[:],
                                         start=True, stop=True)
                        rcp = rcpp.tile([128, 512], f32, name="rcp",
                                        tag="rcp")
                        with nc.allow_low_precision(
                                reason="elementwise reciprocal"):
                            nc.vector.reciprocal(rcp[:], den[:])
                        nc.vector.tensor_mul(
                            outf_c[h][:], outp[:], rcp[:])
                        pull(2)

                    # finish oproj(c-1) + proj(c+1) chains before oproj(c)
                    drain()

                    # oproj(c) chains: filler for attn(c+1) (or drained
                    # at the end for the last chunk)
                    for oc in range(16):
                        filler.append(gen_oproj_chain(c, oc, outf_c))

                drain()

            if reps == 1:
                body()
            else:
                # unroll the rep loop: each For_i iteration carries an
                # all-engine barrier, so amortize it over several bodies
                u = 1
                for cand in (4, 3, 2):
                    if reps % cand == 0:
                        u = cand
                        break
                with tc.For_i(0, reps // u, 1):
                    for _ in range(u):
                        body()

    nc.compile()
    return nc


def get_slopes():
    start = 2 ** (-2 ** (-(math.log2(16) - 3)))
    return np.array([start * start ** i for i in range(16)], np.float32)


def make_host_inputs(x, Wq, bq, Wk, bk, Wv, bv, Wo, bo):
    """Shard full inputs into 8 per-core input maps."""
    nbf16 = mybir.dt.np(bf16)
    x = np.asarray(x, np.float32)
    Wq = np.asarray(Wq, np.float32); bq = np.asarray(bq, np.float32)
    Wk = np.asarray(Wk, np.float32); bk = np.asarray(bk, np.float32)
    Wv = np.asarray(Wv, np.float32)
    Wo = np.asarray(Wo, np.float32)

    slopes = get_slopes()
    sc = np.float32(1.0 / math.sqrt(HD))
    jl = np.arange(128, dtype=np.float32)
    il = np.arange(QT, dtype=np.float32)

    masktri = np.where(jl[:, None] > np.arange(128)[None, :],
                       np.float32(-1e10), np.float32(0.0))
    tri01 = np.where(jl[:, None] > np.arange(128)[None, :],
                     np.float32(0.0), np.float32(1.0))
    ones128 = np.ones((128, 128), np.float32)
    onesrow = np.ones((1, 128), np.float32)
    identb = np.eye(128, dtype=np.float32)

    in_maps = []
    for core in range(8):
        b, s = core // 4, core % 4
        heads = [s, s + 4, s + 8, s + 12]
        cols = np.concatenate(
            [np.arange(h * HD, (h + 1) * HD) for h in heads])
        # bias column for tile (c, jc): anchored at the last query of
        # chunk c:  b[jl] = -slope * (128*(4c - jc) + 511 - jl)
        alibi = np.empty((128, NH * 16), np.float32)
        mshift = np.empty((1, NH * QT), np.float32)
        for hh, h in enumerate(heads):
            for dd in range(16):
                kk = dd - 3  # 4c - jc
                alibi[:, hh * 16 + dd] = -slopes[h] * (
                    128.0 * kk + 511.0 - jl)
            mshift[0, hh * QT:(hh + 1) * QT] = slopes[h] * (511.0 - il)
        bqk = np.zeros((128, 2, NH), np.float32)
        bqk[:, 0, :] = (bq[cols] * sc).reshape(NH, HD).T
        bqk[:, 1, :] = bk[cols].reshape(NH, HD).T
        def pack_x(xb):
            # [emb, t] -> tile-packed [(q*4+g)*128+p, c, t]
            a = xb.reshape(4, 4, 128, 4, 512)          # [g, c, p, q, t]
            return np.ascontiguousarray(
                a.transpose(3, 0, 2, 1, 4).reshape(16 * 128, 4, 512))

        def pack_w(w):
            # [emb, m] -> [(g*128+p), c, m]
            a = w.reshape(4, 4, 128, HG)               # [g, c, p, m]
            return np.ascontiguousarray(
                a.transpose(0, 2, 1, 3).reshape(4 * 128, 4, HG))

        in_maps.append({
            "xq": pack_x(x[b].T).astype(nbf16),
            "wq": pack_w(Wq[:, cols] * sc).astype(nbf16),
            "wk": pack_w(Wk[:, cols]).astype(nbf16),
            "wv": pack_w(Wv[:, cols]).astype(nbf16),
            "wo": np.ascontiguousarray(Wo[cols, :]).astype(nbf16),
            "bqk": bqk,
            "alibi": alibi,
            "mshift": mshift,
            "masktri": masktri.astype(nbf16),
            "ones": ones128,
            "onesrow": onesrow,
            "identb": identb.astype(nbf16),
            "tri01": tri01.astype(nbf16),
        })
    return in_maps


def assemble_output(results, Wv_bias=None, bo=None, Wo=None):
    """results: list of 8 per-core dicts with 'yT' (bf16 partials).

    v-bias folds out of attention exactly: out_h = attn(v'_h) + bv_h,
    so y = sum_h out_h Wo_h = y' + bv @ Wo. Added here with bo.
    Positional-compat: assemble_output(results, bo) treats bv as zero.
    """
    if bo is None:
        Wv_bias, bo = None, Wv_bias
    bo = np.asarray(bo, np.float32)
    if Wv_bias is not None and Wo is not None and np.any(Wv_bias):
        extra = np.asarray(Wv_bias, np.float32) @ np.asarray(Wo, np.float32) + bo
    else:
        extra = bo  # (2048,)
    out = np.empty((2, T, EMB), np.float32)
    for b in range(2):
        acc = np.asarray(results[b * 4 + 0]["yT"], np.float32)
        for s in range(1, 4):
            acc += np.asarray(results[b * 4 + s]["yT"], np.float32)
        # un-pack the pairwise tile layout [c*8+j, 128, 2, 512] back to
        # [emb_out, tok]
        acc = acc.reshape(4, 8, 128, 2, 512).transpose(
            1, 3, 2, 0, 4).reshape(T, T)
        out[b] = acc.T + extra
    return out


class SpmdRunner:
    def __init__(self, nc, n_cores: int):
        install_neuronx_cc_hook()
        self.nc = nc
        self.n_cores = n_cores
        assert nc.dbg_addr is None or not nc.dbg_callbacks
        partition_name = (
            nc.partition_id_tensor.name if nc.partition_id_tensor else None
        )
        in_names, out_names, out_avals = [], [], []
        for alloc in nc.m.functions[0].allocations:
            if not isinstance(alloc, mybir.MemoryLocationSet):
                continue
            name = alloc.memorylocations[0].name
            if alloc.kind == "ExternalInput":
                if name != partition_name:
                    in_names.append(name)
            elif alloc.kind == "ExternalOutput":
                shape = tuple(alloc.tensor_shape)
                dtype = mybir.dt.np(alloc.dtype)
                out_names.append(name)
                out_avals.append(jax.core.ShapedArray(shape, dtype))
        self.in_names = list(in_names)
        self.out_names = out_names
        self.out_avals = out_avals
        n_params = len(self.in_names)
        all_in_names = list(in_names) + list(out_names)
        if partition_name is not None:
            all_in_names.append(partition_name)
        self.partition_name = partition_name

        def _body(*args):
            operands = list(args)
            if partition_name is not None:
                operands.append(bass2jax.partition_id_tensor())
            outs = _bass_exec_p.bind(
                *operands,
                out_avals=tuple(out_avals),
                in_names=tuple(all_in_names),
                out_names=tuple(out_names),
                lowering_input_output_aliases=(),
                sim_require_finite=True,
                sim_require_nnan=True,
                nc=nc,
            )
            return tuple(outs)

        devices = jax.devices()[:n_cores]
        assert len(devices) == n_cores
        self.mesh = Mesh(np.asarray(devices), ("core",))
        n_outs = len(out_names)
        in_specs = (PartitionSpec("core"),) * (n_params + n_outs)
        out_specs = (PartitionSpec("core"),) * n_outs
        self.fn = jax.jit(
            shard_map(_body, mesh=self.mesh, in_specs=in_specs,
                      out_specs=out_specs, check_rep=False),
            keep_unused=True,
        )
        self.dev_args = None

    def set_inputs(self, in_maps: list[dict]):
        """device_put concatenated per-core inputs + zero output buffers."""
        n = self.n_cores
        assert len(in_maps) == n
        concat_in = [
            np.concatenate([np.asarray(in_maps[c][name]) for c in range(n)], axis=0)
            for name in self.in_names
        ]
        concat_zeros = [
            np.zeros((n * a.shape[0], *a.shape[1:]), a.dtype) for a in self.out_avals
        ]
        sharding = jax.sharding.NamedSharding(self.mesh, PartitionSpec("core"))
        self.dev_args = [jax.device_put(a, sharding) for a in concat_in + concat_zeros]

    def run(self):
        outs = self.fn(*self.dev_args)
        jax.block_until_ready(outs)
        return outs

    def results(self, outs) -> list[dict]:
        n = self.n_cores
        return [
            {
                name: np.asarray(outs[i]).reshape(n, *self.out_avals[i].shape)[c]
                for i, name in enumerate(self.out_names)
            }
            for c in range(n)
        ]

    def time_execs(self, iters: int = 10, warmup: int = 2):
        for _ in range(warmup):
            self.run()
        t0 = time.perf_counter()
        for _ in range(iters):
            outs = self.fn(*self.dev_args)
        jax.block_until_ready(outs)
        t1 = time.perf_counter()
        return (t1 - t0) / iters


_RUNNER = None


def _get_runner():
    global _RUNNER
    if _RUNNER is None:
        nc = build_program(reps=1)
        _RUNNER = SpmdRunner(nc, 8)
    return _RUNNER


def kernel(x, Wq, bq, Wk, bk, Wv, bv, Wo, bo):
    r = _get_runner()
    in_maps = make_host_inputs(x, Wq, bq, Wk, bk, Wv, bv, Wo, bo)
    r.set_inputs(in_maps)
    outs = r.run()
    res = r.results(outs)
    return assemble_output(res, bv, bo, Wo)


# revision 30
# speedup vs baseline: 1.0352x; 1.0091x over previous
"""Trainium2 Bass kernel for nn_CausalAttention_84018150244353.

kernel(**inputs) takes the FULL unsharded inputs (as in reference
setup_inputs) and returns the full (2, 2048, 2048) float32 output.

Sharding: 8 NeuronCores = 2 batches x 4 head-slots. Heads are grouped
into 4 work classes by ALiBi slope (large slopes attend only a short
window, so distant key chunks are dropped); each core gets one head of
each class so all cores run the identical program with balanced work:
  slot 0 (heads 0-3):   5 key chunks per query chunk
  slot 1 (heads 4-7):   5 key chunks
  slot 2 (heads 8-11):  8 key chunks
  slot 3 (heads 12-15): full causal
Core (b, s) handles batch b and heads {s, s+4, s+8, s+12}.

Per-core program, software-pipelined so the PE never waits on the
Scalar engine's exp chain:
  proj(0) runs standalone; thereafter the q/k/v projection chains for
  chunk c+1 and the output-projection chains for chunk c-1 are pulled
  as PE "filler" work between attention tiles of chunk c (the exp on
  Scalar is the per-tile rate limiter; two 512-col filler matmuls per
  tile cover the deficit). The exp-sum accumulator (pacc) and the
  PSUM->SBUF copies run on the Vector engine (DVE).
  Host sums the 4 head-slot partials, adds bo + bv @ Wo.
"""
import math
import os
import sys
import time
from collections import deque

sys.path.insert(0, "/opt/trn_rl_repo")

import numpy as np
import jax

jax.config.update("jax_compilation_cache_dir",
                  os.environ.get("JAX_NEFF_CACHE", "/tmp/jax_neff_cache"))
jax.config.update("jax_persistent_cache_min_compile_time_secs", 0.0)
jax.config.update("jax_persistent_cache_min_entry_size_bytes", -1)

from jax.sharding import Mesh, PartitionSpec
from jax.experimental.shard_map import shard_map

import concourse.bass as bass
import concourse.mybir as mybir
import concourse.tile as tile
from concourse import bacc
from concourse import bass2jax
from concourse.bass2jax import _bass_exec_p, install_neuronx_cc_hook

f32 = mybir.dt.float32
r32 = mybir.dt.float32r
bf16 = mybir.dt.bfloat16
Exp = mybir.ActivationFunctionType.Exp
Ident = mybir.ActivationFunctionType.Identity
AddOp = mybir.AluOpType.add

T = 2048
EMB = 2048
HG = 512          # columns per core (4 heads x 128)
HD = 128
NH = 4            # heads per core
NQ = 4            # T quarters
QT = T // NQ      # 512
NE = EMB // 128   # 16 contraction chunks
NJ = T // 128     # 16 key chunks
KS = (5, 5, 8, 16)    # kept key chunks per head slot (ALiBi cutoff)
LOOK = 3


def kept_range(c: int, s: int):
    return range(max(0, 4 * c + 4 - KS[s]), 4 * c + 4)


# Max ALiBi slope of the heads a slot can hold (slot s holds heads
# {s, s+4, s+8, s+12}; classes are h//4).
_SLOT_MAX_SLOPE = (2.0 ** -0.5, 2.0 ** -2.5, 2.0 ** -4.5, 2.0 ** -6.5)


def needs_msh(c: int, s: int) -> bool:
    """Whether exp() needs the rank-1 per-query stabilization shift.
    Slot 1's alibi is anchored at mid-chunk (query 256): with slopes
    <= 2^-2.5 both the overflow side (+slope*255 + score ~ e^53) and the
    underflow side (-slope*384 - score ~ e^-76) stay in fp32/bf16 normal
    range, so only slot 0 (slopes up to 2^-0.5) needs the shift. Slots
    2/3 stay end-anchored with floors above e^-70."""
    if s == 1:
        return False
    depth = min(512 * c + 511, 511 + 128 * (KS[s] - 4))
    return _SLOT_MAX_SLOPE[s] * depth > 70.0


def build_program(reps: int = 1, sim_safe: bool = False):
    """sim_safe=True keeps start-matmuls full-width so exec-mode CoreSim's
    PSUM pending-zero bookkeeping holds (hardware is fine either way)."""
    nc = bacc.Bacc("TRN2", target_bir_lowering=False, debug=False,
                   enable_asserts=False, num_devices=8)

    # x and w are tile-packed host-side so every load is fully
    # contiguous per partition (4KB descriptor lines instead of 1KB)
    xT_d = nc.dram_tensor("xq", [NQ * 4 * 128, 4, QT], bf16,
                          kind="ExternalInput")
    wq_d = nc.dram_tensor("wq", [4 * 128, 4, HG], bf16, kind="ExternalInput")
    wk_d = nc.dram_tensor("wk", [4 * 128, 4, HG], bf16, kind="ExternalInput")
    wv_d = nc.dram_tensor("wv", [4 * 128, 4, HG], bf16, kind="ExternalInput")
    wo_d = nc.dram_tensor("wo", [HG, T], bf16, kind="ExternalInput")
    bqk_d = nc.dram_tensor("bqk", [128, 2, NH], f32, kind="ExternalInput")
    # per-(head, 4c-jc) re-anchored ALiBi bias columns
    alibi_d = nc.dram_tensor("alibi", [128, NH * 16], f32, kind="ExternalInput")
    # per-(head, query-in-chunk) stabilization row, added rank-1
    mshift_d = nc.dram_tensor("mshift", [1, NH * QT], r32, kind="ExternalInput")
    # triangular causal mask for the 128x128 block at the tile diagonal
    masktri_d = nc.dram_tensor("masktri", [128, 128], bf16, kind="ExternalInput")
    ones_d = nc.dram_tensor("ones", [128, 128], r32, kind="ExternalInput")
    onesrow_d = nc.dram_tensor("onesrow", [1, 128], r32, kind="ExternalInput")
    identb_d = nc.dram_tensor("identb", [128, 128], bf16, kind="ExternalInput")
    tri01_d = nc.dram_tensor("tri01", [128, 128], bf16, kind="ExternalInput")
    # output tiles packed pairwise: [c*8 + oc//2] -> [128, 1024]
    yT_d = nc.dram_tensor("yT", [NQ * 8 * 128, 1024], bf16,
                          kind="ExternalOutput")

    with tile.TileContext(nc) as tc:
        with (
            tc.tile_pool(name="consts", bufs=1) as consts,
            tc.tile_pool(name="wslab", bufs=1) as wslab,
            tc.tile_pool(name="qkvp", bufs=1) as qkvp,
            tc.tile_pool(name="xp", bufs=3) as xp,
            tc.tile_pool(name="outfp", bufs=2) as outfp,
            tc.tile_pool(name="pp", bufs=4) as pp,
            tc.tile_pool(name="rcpp", bufs=2) as rcpp,
            tc.tile_pool(name="paccp", bufs=2) as paccp,
            tc.tile_pool(name="ysb", bufs=6) as ysb,
            tc.tile_pool(name="ps_p1", bufs=2, space="PSUM") as ps_p1,
            tc.tile_pool(name="ps_s", bufs=3, space="PSUM") as ps_s,
            tc.tile_pool(name="ps_o", bufs=2, space="PSUM") as ps_o,
            tc.tile_pool(name="ps_d", bufs=1, space="PSUM") as ps_d,
        ):
            def xq_src(q, g):
                i = (q * 4 + g) * 128
                return xT_d.ap()[i:i + 128]

            def w_src(w_d, g):
                return w_d.ap()[g * 128:(g + 1) * 128]

            def body():
                # ---- startup loads: one list in need-order, issued
                # round-robin across the 3 DMA-capable queues so each
                # tensor lands roughly when its first consumer runs ----
                x_t = [[None] * 4 for _ in range(NQ)]
                wq_p, wk_p, wv_p = [], [], []
                bqk_sb = consts.tile([128, 2, NH], f32, name="bqk_sb")
                alibi_sb = consts.tile([128, NH * 16], f32, name="alibi_sb")
                masktri_sb = consts.tile([128, 128], bf16, name="masktri_sb")
                ones_sb = consts.tile([128, 128], r32, name="ones_sb")
                onesr_sb = consts.tile([1, 128], r32, name="onesr_sb")
                identb_sb = consts.tile([128, 128], bf16, name="identb_sb")
                tri01_sb = consts.tile([128, 128], bf16, name="tri01_sb")
                msh_sb = consts.tile([1, NH * QT], r32, name="msh_sb")
                wo_sb = wslab.tile([128, NH, T], bf16, name="wo_sb")
                # part g=0 of x chunk 0 and of wq lives in two half
                # tiles so the first matmuls wait on a ~0.7us DMA pair,
                # not the whole 512KB parts (readers wait on all writers
                # of a tile, so sub-tile DMA splits don't help)
                x0h = [wslab.tile([128, 1, QT], bf16, name=f"x0h{i}")
                       for i in range(4)]
                wq0h = [wslab.tile([128, 1, HG], bf16, name=f"wq0h{i}")
                        for i in range(4)]
                for g in range(4):
                    if g > 0:
                        x_t[0][g] = xp.tile([128, 4, QT], bf16,
                                            name=f"x_p{g}", tag=f"x_p{g}")
                    x_t[1][g] = xp.tile([128, 4, QT], bf16, name=f"x_p{g}",
                                        tag=f"x_p{g}")
                    wq_p.append(None if g == 0 else
                                wslab.tile([128, 4, HG], bf16,
                                           name=f"wq_p{g}"))
                    wk_p.append(wslab.tile([128, 4, HG], bf16,
                                           name=f"wk_p{g}"))
                    wv_p.append(wslab.tile([128, 4, HG], bf16,
                                           name=f"wv_p{g}"))

                def x0_sl(e, cols=slice(None)):
                    if e < 4:
                        return x0h[e][:, 0, cols]
                    return x_t[0][e // 4][:, e % 4, cols]

                def wq_sl(e, cols):
                    if e < 4:
                        return wq0h[e][:, 0, cols]
                    return wq_p[e // 4][:, e % 4, cols]

                wo_r = wo_d.ap().rearrange("(h p) o -> p h o", p=128)
                dmas = []  # (dst, src) in first-use order
                for g in range(4):
                    if g == 0:
                        for e in range(4):
                            dmas.append((x0h[e][:],
                                         xq_src(0, 0)[:, e:e + 1, :]))
                            dmas.append((wq0h[e][:],
                                         w_src(wq_d, 0)[:, e:e + 1, :]))
                            if e == 0:
                                dmas.append((bqk_sb[:], bqk_d.ap()))
                        continue
                    dmas.append((x_t[0][g][:], xq_src(0, g)))
                    dmas.append((wq_p[g][:], w_src(wq_d, g)))
                for g in range(4):
                    dmas.append((wk_p[g][:], w_src(wk_d, g)))
                dmas += [(alibi_sb[:], alibi_d.ap()),
                         (masktri_sb[:], masktri_d.ap()),
                         (ones_sb[:], ones_d.ap()),
                         (onesr_sb[:], onesrow_d.ap()),
                         (identb_sb[:], identb_d.ap()),
                         (tri01_sb[:], tri01_d.ap()),
                         (msh_sb[:], mshift_d.ap())]
                for g in range(4):
                    dmas.append((wv_p[g][:], w_src(wv_d, g)))
                # x chunk 1 prefetch (needed once proj(1) filler chains
                # start, well into attn(0)); wo only at the oproj(0)
                # fillers during attn(1)
                for g in range(4):
                    dmas.append((x_t[1][g][:], xq_src(1, g)))
                for h in range(NH):
                    dmas.append((wo_sb[:, h, :], wo_r[:, h, :]))
                qs = (nc.sync, nc.scalar, nc.gpsimd)
                for i, (dst, src) in enumerate(dmas):
                    qs[i % 3].dma_start(dst, src)

                qT_sb = [qkvp.tile([128, NH, QT], bf16, name=f"qT_sb{q}")
                         for q in range(NQ)]
                kT_sb = [qkvp.tile([128, NH, QT], bf16, name=f"kT_sb{q}")
                         for q in range(NQ)]
                v_sb = [qkvp.tile([128, 4, HG], bf16, name=f"v_sb{q}")
                        for q in range(NQ)]

                # ---- PE filler machinery: projection / output chains as
                # generators, one matmul emitted per pull ----
                def gen_qk_chain(c, pi, cc):
                    dst = (qT_sb, kT_sb)[pi]
                    csl = slice(cc * 128, (cc + 1) * 128)
                    ps = ps_p1.tile([128, 512], f32, name="p1acc",
                                    tag="p1acc")
                    for e in range(NE):
                        w_ap = (wq_sl(e, csl) if pi == 0 else
                                wk_p[e // 4][:, e % 4, csl])
                        nc.tensor.matmul(
                            ps[:], w_ap,
                            x_t[c][e // 4][:, e % 4, :],
                            start=(e == 0), stop=(e == NE - 1))
                        if e < NE - 1:
                            yield
                    nc.scalar.activation(
                        dst[c][:, cc, :], ps[:], Ident,
                        bias=bqk_sb[:, pi, cc:cc + 1])
                    yield

                def gen_v_chain(c, tb):
                    ps = ps_p1.tile([128, 512], f32, name="p1acc",
                                    tag="p1acc")
                    for e in range(NE):
                        nc.tensor.matmul(
                            ps[:],
                            x_t[c][e // 4][:, e % 4, tb * 128:(tb + 1) * 128],
                            wv_p[e // 4][:, e % 4, :],
                            start=(e == 0), stop=(e == NE - 1))
                        if e < NE - 1:
                            yield
                    nc.vector.tensor_copy(v_sb[c][:, tb, :], ps[:])
                    yield

                ys_pairs = {}

                def gen_oproj_chain(c, oc, outf_c):
                    if c == NQ - 1:
                        # final drain: attention is done, so the score /
                        # out PSUM pools and all copy engines are free —
                        # rotate across them so the 4-matmul chains never
                        # wait on a PSUM->SBUF copy
                        pool, ptag = [(ps_p1, "p1acc"), (ps_s, "s_ps"),
                                      (ps_o, "out_ps")][oc % 3]
                    else:
                        pool, ptag = ps_p1, "p1acc"
                    yp = pool.tile([128, 512], f32, name="y_ps", tag=ptag)
                    hord = (3, 2, 1, 0) if c == NQ - 1 else range(NH)
                    for hi, h in enumerate(hord):
                        nc.tensor.matmul(
                            yp[:],
                            wo_sb[:, h, oc * 128:(oc + 1) * 128],
                            outf_c[h][:],
                            start=(hi == 0), stop=(hi == 3))
                        if hi < NH - 1:
                            yield
                    # adjacent oc share one [128, 1024] SBUF tile so the
                    # store is a single 2KB-per-partition-line DMA
                    if oc % 2 == 0:
                        ys = ysb.tile([128, 1024], bf16, name="y_sb",
                                      tag="y_sb")
                        ys_pairs[c] = ys
                        nc.vector.tensor_copy(ys[:, 0:512], yp[:])
                        yield
                        return
                    ys = ys_pairs.pop(c)
                    nc.scalar.copy(ys[:, 512:1024], yp[:])
                    blk = (c * 8 + oc // 2) * 128
                    dst = yT_d.ap()[blk:blk + 128, :]
                    qeng = ([nc.gpsimd, nc.scalar, nc.sync][(oc // 2) % 3]
                            if c == NQ - 1 else
                            [nc.gpsimd, nc.scalar][(oc // 2) % 2])
                    qeng.dma_start(dst, ys[:])
                    yield

                filler = deque()

                def pull(n):
                    while n > 0 and filler:
                        try:
                            next(filler[0])
                            n -= 1
                        except StopIteration:
                            filler.popleft()

                def drain():
                    while filler:
                        try:
                            next(filler[0])
                        except StopIteration:
                            filler.popleft()

                def enqueue_proj(c):
                    for cc in range(4):
                        filler.append(gen_qk_chain(c, 0, cc))
                    for cc in range(4):
                        filler.append(gen_qk_chain(c, 1, cc))
                    for tb in range(4):
                        filler.append(gen_v_chain(c, tb))

                # ============ proj(0): standalone ============
                # Part-major with 4 parallel PSUM accumulators (the
                # score/out/den pools are all free at startup), so the PE
                # consumes each x/w part as its DMA lands instead of
                # serializing whole chains behind the 2-buffer p1 pool.
                def proj0_phase(accs, stat_of, mov_of, emit_copy):
                    for e in range(NE):
                        for j in range(4):
                            nc.tensor.matmul(
                                accs[j][:], stat_of(e, j), mov_of(e, j),
                                start=(e == 0), stop=(e == NE - 1))
                    for j in range(4):
                        emit_copy(j, accs[j])

                qk_accs = lambda: [
                    ps_p1.tile([128, 512], f32, name="p1acc", tag="p1acc"),
                    ps_p1.tile([128, 512], f32, name="p1acc", tag="p1acc"),
                    ps_s.tile([128, 512], f32, name="s_ps", tag="s_ps"),
                    ps_s.tile([128, 512], f32, name="s_ps", tag="s_ps"),
                ]
                proj0_phase(
                    qk_accs(),
                    lambda e, cc: wq_sl(e, slice(cc * 128, (cc + 1) * 128)),
                    lambda e, cc: x0_sl(e),
                    lambda cc, ps: nc.scalar.activation(
                        qT_sb[0][:, cc, :], ps[:], Ident,
                        bias=bqk_sb[:, 0, cc:cc + 1]))
                proj0_phase(
                    [ps_o.tile([128, 512], f32, name="out_ps", tag="out_ps"),
                     ps_o.tile([128, 512], f32, name="out_ps", tag="out_ps"),
                     ps_s.tile([128, 512], f32, name="s_ps", tag="s_ps"),
                     ps_d.tile([128, 512], f32, name="den_ps", tag="den_ps")],
                    lambda e, cc: wk_p[e // 4][:, e % 4,
                                               cc * 128:(cc + 1) * 128],
                    lambda e, cc: x0_sl(e),
                    lambda cc, ps: nc.scalar.activation(
                        kT_sb[0][:, cc, :], ps[:], Ident,
                        bias=bqk_sb[:, 1, cc:cc + 1]))
                proj0_phase(
                    qk_accs(),
                    lambda e, tb: x0_sl(e, slice(tb * 128, (tb + 1) * 128)),
                    lambda e, tb: wv_p[e // 4][:, e % 4, :],
                    lambda tb, ps: nc.vector.tensor_copy(
                        v_sb[0][:, tb, :], ps[:]))

                for c in range(NQ):
                    # x for chunk c+2 loads during attn(c) (x1 was loaded
                    # at startup); needed by proj(c+2) fillers in attn(c+1)
                    if 2 <= c + 2 < NQ:
                        for g in range(4):
                            x_t[c + 2][g] = xp.tile(
                                [128, 4, QT], bf16,
                                name=f"x_p{g}", tag=f"x_p{g}")
                            nc.sync.dma_start(
                                x_t[c + 2][g][:], xq_src(c + 2, g))

                    # proj(c+1) chains become filler for attn(c), behind
                    # any oproj(c-1) chains still queued
                    if c + 1 < NQ:
                        enqueue_proj(c + 1)

                    # ================ attn(c) ================
                    # per-head outf tiles keep oproj from waiting on the
                    # later heads' normalization tails
                    outf_c = [outfp.tile([128, 512], bf16,
                                         name=f"outf_h{h}", tag=f"outf_h{h}")
                              for h in range(NH)]
                    # last chunk: biggest head first so oproj(NQ-1) chains
                    # (accumulated in the same order) start while the
                    # small heads' softmax tails are still normalizing
                    horder = (3, 2, 1, 0) if c == NQ - 1 else range(NH)
                    for h in horder:
                        msh = needs_msh(c, h)
                        kept = list(kept_range(c, h))
                        first, last = kept[0], kept[-1]

                        s_tiles = {}
                        p_tiles = {}

                        def qlo_of(jc):
                            # queries below 128*(jc-4c) see no valid key
                            # in diag tile jc: skip that region entirely
                            return max(0, (jc - 4 * c) * 128)

                        def emit_score(jc):
                            # causal mask (one 128x128 triangle) and the
                            # rank-1 stabilization shift accumulate onto
                            # the score PSUM on the PE itself.
                            # The start matmul is always full-width so the
                            # PSUM bank has a uniform accumulation state.
                            s = ps_s.tile([128, 512], f32, name="s_ps",
                                          tag="s_ps")
                            diag = jc >= 4 * c
                            dlo = qlo_of(jc)
                            slo = 0 if sim_safe else dlo
                            mask_mm = diag and msh
                            nc.tensor.matmul(
                                s[:, slo:],
                                kT_sb[jc // 4][:, h,
                                               (jc % 4) * 128:
                                               (jc % 4 + 1) * 128],
                                qT_sb[c][:, h, slo:],
                                start=True, stop=not (mask_mm or msh))
                            if mask_mm:
                                nc.tensor.matmul(
                                    s[:, dlo:dlo + 128],
                                    identb_sb[:], masktri_sb[:],
                                    start=False, stop=not msh)
                            if msh:
                                mlo = min(dlo, 256)
                                nc.tensor.matmul(
                                    s[:, mlo:],
                                    onesr_sb[:],
                                    msh_sb[0:1, h * QT + mlo:
                                           (h + 1) * QT],
                                    start=False, stop=True)
                            s_tiles[jc] = s

                        def emit_exp(jc):
                            qlo = qlo_of(jc)
                            p = pp.tile([128, 512], bf16, name="p_sb",
                                        tag="p_sb")
                            nc.scalar.activation(
                                p[:, qlo:], s_tiles.pop(jc)[:, qlo:], Exp,
                                bias=alibi_sb[:, h * 16 + 4 * c - jc + 3:
                                              h * 16 + 4 * c - jc + 4])
                            if jc >= 4 * c and not msh:
                                # causal mask applied on p just ahead of
                                # the pacc add in the DVE FIFO; saves the
                                # PSUM mask matmul on the PE
                                nc.vector.tensor_tensor(
                                    p[:, qlo:qlo + 128],
                                    p[:, qlo:qlo + 128], tri01_sb[:],
                                    mybir.AluOpType.mult)
                            p_tiles[jc] = p

                        outp = ps_o.tile([128, 512], f32, name="out_ps",
                                         tag="out_ps")
                        # p accumulates on DVE (SBUF only); den is one
                        # matmul per head instead of one per tile
                        pacc = paccp.tile([128, 512], r32, name="pacc_sb",
                                          tag="pacc_sb")

                        def emit_consume(jc):
                            p = p_tiles.pop(jc)
                            qlo = qlo_of(jc)
                            if jc == first:
                                nc.vector.tensor_copy(pacc[:], p[:])
                            else:
                                nc.vector.tensor_tensor(
                                    pacc[:, qlo:], pacc[:, qlo:],
                                    p[:, qlo:], AddOp)
                            nc.tensor.matmul(
                                outp[:, qlo:],
                                v_sb[jc // 4][:, jc % 4,
                                              h * 128:(h + 1) * 128],
                                p[:, qlo:],
                                start=(jc == first), stop=(jc == last))

                        for i in range(min(LOOK, len(kept))):
                            emit_score(kept[i])
                        for i, jc in enumerate(kept):
                            if i + LOOK < len(kept):
                                emit_score(kept[i + LOOK])
                            pull(2)
                            emit_exp(jc)
                            emit_consume(jc)

                        # filler before den hides the DVE pacc tail
                        pull(3)
                        den = ps_d.tile([128, 512], f32, name="den_ps",
                                        tag="den_ps")
                        nc.tensor.matmul(den[:], ones_sb[:], pacc[:],
                                         start=True, stop=True)
                        rcp = rcpp.tile([128, 512], f32, name="rcp",
                                        tag="rcp")
                        with nc.allow_low_precision(
                                reason="elementwise reciprocal"):
                            nc.vector.reciprocal(rcp[:], den[:])
                        nc.vector.tensor_mul(
                            outf_c[h][:], outp[:], rcp[:])
                        pull(2)

                    # finish oproj(c-1) + proj(c+1) chains before oproj(c)
                    drain()

                    # oproj(c) chains: filler for attn(c+1) (or drained
                    # at the end for the last chunk)
                    for oc in range(16):
                        filler.append(gen_oproj_chain(c, oc, outf_c))

                drain()

            if reps == 1:
                body()
            else:
                # unroll the rep loop: each For_i iteration carries an
                # all-engine barrier, so amortize it over several bodies
                u = 1
                for cand in (4, 3, 2):
                    if reps % cand == 0:
                        u = cand
                        break
                with tc.For_i(0, reps // u, 1):
                    for _ in range(u):
                        body()

    nc.compile()
    return nc


def get_slopes():
    start = 2 ** (-2 ** (-(math.log2(16) - 3)))
    return np.array([start * start ** i for i in range(16)], np.float32)


def make_host_inputs(x, Wq, bq, Wk, bk, Wv, bv, Wo, bo):
    """Shard full inputs into 8 per-core input maps."""
    nbf16 = mybir.dt.np(bf16)
    x = np.asarray(x, np.float32)
    Wq = np.asarray(Wq, np.float32); bq = np.asarray(bq, np.float32)
    Wk = np.asarray(Wk, np.float32); bk = np.asarray(bk, np.float32)
    Wv = np.asarray(Wv, np.float32)
    Wo = np.asarray(Wo, np.float32)

    slopes = get_slopes()
    sc = np.float32(1.0 / math.sqrt(HD))
    jl = np.arange(128, dtype=np.float32)
    il = np.arange(QT, dtype=np.float32)

    masktri = np.where(jl[:, None] > np.arange(128)[None, :],
                       np.float32(-1e10), np.float32(0.0))
    tri01 = np.where(jl[:, None] > np.arange(128)[None, :],
                     np.float32(0.0), np.float32(1.0))
    ones128 = np.ones((128, 128), np.float32)
    onesrow = np.ones((1, 128), np.float32)
    identb = np.eye(128, dtype=np.float32)

    in_maps = []
    for core in range(8):
        b, s = core // 4, core % 4
        heads = [s, s + 4, s + 8, s + 12]
        cols = np.concatenate(
            [np.arange(h * HD, (h + 1) * HD) for h in heads])
        # bias column for tile (c, jc): anchored at the last query of
        # chunk c:  b[jl] = -slope * (128*(4c - jc) + 511 - jl)
        alibi = np.empty((128, NH * 16), np.float32)
        mshift = np.empty((1, NH * QT), np.float32)
        for hh, h in enumerate(heads):
            anchor = 256.0 if hh == 1 else 511.0
            for dd in range(16):
                kk = dd - 3  # 4c - jc
                alibi[:, hh * 16 + dd] = -slopes[h] * (
                    128.0 * kk + anchor - jl)
            mshift[0, hh * QT:(hh + 1) * QT] = slopes[h] * (511.0 - il)
        bqk = np.zeros((128, 2, NH), np.float32)
        bqk[:, 0, :] = (bq[cols] * sc).reshape(NH, HD).T
        bqk[:, 1, :] = bk[cols].reshape(NH, HD).T
        def pack_x(xb):
            # [emb, t] -> tile-packed [(q*4+g)*128+p, c, t]
            a = xb.reshape(4, 4, 128, 4, 512)          # [g, c, p, q, t]
            return np.ascontiguousarray(
                a.transpose(3, 0, 2, 1, 4).reshape(16 * 128, 4, 512))

        def pack_w(w):
            # [emb, m] -> [(g*128+p), c, m]
            a = w.reshape(4, 4, 128, HG)               # [g, c, p, m]
            return np.ascontiguousarray(
                a.transpose(0, 2, 1, 3).reshape(4 * 128, 4, HG))

        in_maps.append({
            "xq": pack_x(x[b].T).astype(nbf16),
            "wq": pack_w(Wq[:, cols] * sc).astype(nbf16),
            "wk": pack_w(Wk[:, cols]).astype(nbf16),
            "wv": pack_w(Wv[:, cols]).astype(nbf16),
            "wo": np.ascontiguousarray(Wo[cols, :]).astype(nbf16),
            "bqk": bqk,
            "alibi": alibi,
            "mshift": mshift,
            "masktri": masktri.astype(nbf16),
            "ones": ones128,
            "onesrow": onesrow,
            "identb": identb.astype(nbf16),
            "tri01": tri01.astype(nbf16),
        })
    return in_maps


def assemble_output(results, Wv_bias=None, bo=None, Wo=None):
    """results: list of 8 per-core dicts with 'yT' (bf16 partials).

    v-bias folds out of attention exactly: out_h = attn(v'_h) + bv_h,
    so y = sum_h out_h Wo_h = y' + bv @ Wo. Added here with bo.
    Positional-compat: assemble_output(results, bo) treats bv as zero.
    """
    if bo is None:
        Wv_bias, bo = None, Wv_bias
    bo = np.asarray(bo, np.float32)
    if Wv_bias is not None and Wo is not None and np.any(Wv_bias):
        extra = np.asarray(Wv_bias, np.float32) @ np.asarray(Wo, np.float32) + bo
    else:
        extra = bo  # (2048,)
    out = np.empty((2, T, EMB), np.float32)
    for b in range(2):
        acc = np.asarray(results[b * 4 + 0]["yT"], np.float32)
        for s in range(1, 4):
            acc += np.asarray(results[b * 4 + s]["yT"], np.float32)
        # un-pack the pairwise tile layout [c*8+j, 128, 2, 512] back to
        # [emb_out, tok]
        acc = acc.reshape(4, 8, 128, 2, 512).transpose(
            1, 3, 2, 0, 4).reshape(T, T)
        out[b] = acc.T + extra
    return out


class SpmdRunner:
    def __init__(self, nc, n_cores: int):
        install_neuronx_cc_hook()
        self.nc = nc
        self.n_cores = n_cores
        assert nc.dbg_addr is None or not nc.dbg_callbacks
        partition_name = (
            nc.partition_id_tensor.name if nc.partition_id_tensor else None
        )
        in_names, out_names, out_avals = [], [], []
        for alloc in nc.m.functions[0].allocations:
            if not isinstance(alloc, mybir.MemoryLocationSet):
                continue
            name = alloc.memorylocations[0].name
            if alloc.kind == "ExternalInput":
                if name != partition_name:
                    in_names.append(name)
            elif alloc.kind == "ExternalOutput":
                shape = tuple(alloc.tensor_shape)
                dtype = mybir.dt.np(alloc.dtype)
                out_names.append(name)
                out_avals.append(jax.core.ShapedArray(shape, dtype))
        self.in_names = list(in_names)
        self.out_names = out_names
        self.out_avals = out_avals
        n_params = len(self.in_names)
        all_in_names = list(in_names) + list(out_names)
        if partition_name is not None:
            all_in_names.append(partition_name)
        self.partition_name = partition_name

        def _body(*args):
            operands = list(args)
            if partition_name is not None:
                operands.append(bass2jax.partition_id_tensor())
            outs = _bass_exec_p.bind(
                *operands,
                out_avals=tuple(out_avals),
                in_names=tuple(all_in_names),
                out_names=tuple(out_names),
                lowering_input_output_aliases=(),
                sim_require_finite=True,
                sim_require_nnan=True,
                nc=nc,
            )
            return tuple(outs)

        devices = jax.devices()[:n_cores]
        assert len(devices) == n_cores
        self.mesh = Mesh(np.asarray(devices), ("core",))
        n_outs = len(out_names)
        in_specs = (PartitionSpec("core"),) * (n_params + n_outs)
        out_specs = (PartitionSpec("core"),) * n_outs
        self.fn = jax.jit(
            shard_map(_body, mesh=self.mesh, in_specs=in_specs,
                      out_specs=out_specs, check_rep=False),
            keep_unused=True,
        )
        self.dev_args = None

    def set_inputs(self, in_maps: list[dict]):
        """device_put concatenated per-core inputs + zero output buffers."""
        n = self.n_cores
        assert len(in_maps) == n
        concat_in = [
            np.concatenate([np.asarray(in_maps[c][name]) for c in range(n)], axis=0)
            for name in self.in_names
        ]
        concat_zeros = [
            np.zeros((n * a.shape[0], *a.shape[1:]), a.dtype) for a in self.out_avals
        ]
        sharding = jax.sharding.NamedSharding(self.mesh, PartitionSpec("core"))
        self.dev_args = [jax.device_put(a, sharding) for a in concat_in + concat_zeros]

    def run(self):
        outs = self.fn(*self.dev_args)
        jax.block_until_ready(outs)
        return outs

    def results(self, outs) -> list[dict]:
        n = self.n_cores
        return [
            {
                name: np.asarray(outs[i]).reshape(n, *self.out_avals[i].shape)[c]
                for i, name in enumerate(self.out_names)
            }
            for c in range(n)
        ]

    def time_execs(self, iters: int = 10, warmup: int = 2):
        for _ in range(warmup):
            self.run()
        t0 = time.perf_counter()
        for _ in range(iters):
            outs = self.fn(*self.dev_args)
        jax.block_until_ready(outs)
        t1 = time.perf_counter()
        return (t1 - t0) / iters


_RUNNER = None


def _get_runner():
    global _RUNNER
    if _RUNNER is None:
        nc = build_program(reps=1)
        _RUNNER = SpmdRunner(nc, 8)
    return _RUNNER


def kernel(x, Wq, bq, Wk, bk, Wv, bv, Wo, bo):
    r = _get_runner()
    in_maps = make_host_inputs(x, Wq, bq, Wk, bk, Wv, bv, Wo, bo)
    r.set_inputs(in_maps)
    outs = r.run()
    res = r.results(outs)
    return assemble_output(res, bv, bo, Wo)


# revision 32
# speedup vs baseline: 1.0357x; 1.0005x over previous
"""Trainium2 Bass kernel for nn_CausalAttention_84018150244353.

kernel(**inputs) takes the FULL unsharded inputs (as in reference
setup_inputs) and returns the full (2, 2048, 2048) float32 output.

Sharding: 8 NeuronCores = 2 batches x 4 head-slots. Heads are grouped
into 4 work classes by ALiBi slope (large slopes attend only a short
window, so distant key chunks are dropped); each core gets one head of
each class so all cores run the identical program with balanced work:
  slot 0 (heads 0-3):   5 key chunks per query chunk
  slot 1 (heads 4-7):   5 key chunks
  slot 2 (heads 8-11):  8 key chunks
  slot 3 (heads 12-15): full causal
Core (b, s) handles batch b and heads {s, s+4, s+8, s+12}.

Per-core program, software-pipelined so the PE never waits on the
Scalar engine's exp chain:
  proj(0) runs standalone; thereafter the q/k/v projection chains for
  chunk c+1 and the output-projection chains for chunk c-1 are pulled
  as PE "filler" work between attention tiles of chunk c (the exp on
  Scalar is the per-tile rate limiter; two 512-col filler matmuls per
  tile cover the deficit). The exp-sum accumulator (pacc) and the
  PSUM->SBUF copies run on the Vector engine (DVE).
  Host sums the 4 head-slot partials, adds bo + bv @ Wo.
"""
import math
import os
import sys
import time
from collections import deque

sys.path.insert(0, "/opt/trn_rl_repo")

import numpy as np
import jax

jax.config.update("jax_compilation_cache_dir",
                  os.environ.get("JAX_NEFF_CACHE", "/tmp/jax_neff_cache"))
jax.config.update("jax_persistent_cache_min_compile_time_secs", 0.0)
jax.config.update("jax_persistent_cache_min_entry_size_bytes", -1)

from jax.sharding import Mesh, PartitionSpec
from jax.experimental.shard_map import shard_map

import concourse.bass as bass
import concourse.mybir as mybir
import concourse.tile as tile
from concourse import bacc
from concourse import bass2jax
from concourse.bass2jax import _bass_exec_p, install_neuronx_cc_hook

f32 = mybir.dt.float32
r32 = mybir.dt.float32r
bf16 = mybir.dt.bfloat16
Exp = mybir.ActivationFunctionType.Exp
Ident = mybir.ActivationFunctionType.Identity
AddOp = mybir.AluOpType.add

T = 2048
EMB = 2048
HG = 512          # columns per core (4 heads x 128)
HD = 128
NH = 4            # heads per core
NQ = 4            # T quarters
QT = T // NQ      # 512
NE = EMB // 128   # 16 contraction chunks
NJ = T // 128     # 16 key chunks
KS = (5, 5, 8, 16)    # kept key chunks per head slot (ALiBi cutoff)
LOOK = 3


def kept_range(c: int, s: int):
    return range(max(0, 4 * c + 4 - KS[s]), 4 * c + 4)


# Max ALiBi slope of the heads a slot can hold (slot s holds heads
# {s, s+4, s+8, s+12}; classes are h//4).
_SLOT_MAX_SLOPE = (2.0 ** -0.5, 2.0 ** -2.5, 2.0 ** -4.5, 2.0 ** -6.5)


def needs_msh(c: int, s: int) -> bool:
    """Whether exp() needs the rank-1 per-query stabilization shift.
    Slot 1's alibi is anchored at mid-chunk (query 256): with slopes
    <= 2^-2.5 both the overflow side (+slope*255 + score ~ e^53) and the
    underflow side (-slope*384 - score ~ e^-76) stay in fp32/bf16 normal
    range, so only slot 0 (slopes up to 2^-0.5) needs the shift. Slots
    2/3 stay end-anchored with floors above e^-70."""
    if s == 1:
        return False
    depth = min(512 * c + 511, 511 + 128 * (KS[s] - 4))
    return _SLOT_MAX_SLOPE[s] * depth > 70.0


def build_program(reps: int = 1, sim_safe: bool = False):
    """sim_safe=True keeps start-matmuls full-width so exec-mode CoreSim's
    PSUM pending-zero bookkeeping holds (hardware is fine either way)."""
    nc = bacc.Bacc("TRN2", target_bir_lowering=False, debug=False,
                   enable_asserts=False, num_devices=8)

    # x and w are tile-packed host-side so every load is fully
    # contiguous per partition (4KB descriptor lines instead of 1KB)
    xT_d = nc.dram_tensor("xq", [NQ * 4 * 128, 4, QT], bf16,
                          kind="ExternalInput")
    wq_d = nc.dram_tensor("wq", [4 * 128, 4, HG], bf16, kind="ExternalInput")
    wk_d = nc.dram_tensor("wk", [4 * 128, 4, HG], bf16, kind="ExternalInput")
    wv_d = nc.dram_tensor("wv", [4 * 128, 4, HG], bf16, kind="ExternalInput")
    wo_d = nc.dram_tensor("wo", [HG, T], bf16, kind="ExternalInput")
    bqk_d = nc.dram_tensor("bqk", [128, 2, NH], f32, kind="ExternalInput")
    # per-(head, 4c-jc) re-anchored ALiBi bias columns
    alibi_d = nc.dram_tensor("alibi", [128, NH * 16], f32, kind="ExternalInput")
    # per-(head, query-in-chunk) stabilization row, added rank-1
    mshift_d = nc.dram_tensor("mshift", [1, NH * QT], r32, kind="ExternalInput")
    # triangular causal mask for the 128x128 block at the tile diagonal
    masktri_d = nc.dram_tensor("masktri", [128, 128], bf16, kind="ExternalInput")
    ones_d = nc.dram_tensor("ones", [128, 128], r32, kind="ExternalInput")
    onesrow_d = nc.dram_tensor("onesrow", [1, 128], r32, kind="ExternalInput")
    identb_d = nc.dram_tensor("identb", [128, 128], bf16, kind="ExternalInput")
    tri01_d = nc.dram_tensor("tri01", [128, 128], bf16, kind="ExternalInput")
    # output tiles packed pairwise: [c*8 + oc//2] -> [128, 1024]
    yT_d = nc.dram_tensor("yT", [NQ * 8 * 128, 1024], bf16,
                          kind="ExternalOutput")

    with tile.TileContext(nc) as tc:
        with (
            tc.tile_pool(name="consts", bufs=1) as consts,
            tc.tile_pool(name="wslab", bufs=1) as wslab,
            tc.tile_pool(name="qkvp", bufs=1) as qkvp,
            tc.tile_pool(name="xp", bufs=3) as xp,
            tc.tile_pool(name="outfp", bufs=2) as outfp,
            tc.tile_pool(name="pp", bufs=4) as pp,
            tc.tile_pool(name="rcpp", bufs=2) as rcpp,
            tc.tile_pool(name="paccp", bufs=2) as paccp,
            tc.tile_pool(name="ysb", bufs=6) as ysb,
            tc.tile_pool(name="ps_p1", bufs=2, space="PSUM") as ps_p1,
            tc.tile_pool(name="ps_s", bufs=3, space="PSUM") as ps_s,
            tc.tile_pool(name="ps_o", bufs=2, space="PSUM") as ps_o,
            tc.tile_pool(name="ps_d", bufs=1, space="PSUM") as ps_d,
        ):
            def xq_src(q, g):
                i = (q * 4 + g) * 128
                return xT_d.ap()[i:i + 128]

            def w_src(w_d, g):
                return w_d.ap()[g * 128:(g + 1) * 128]

            def body():
                # ---- startup loads: one list in need-order, issued
                # round-robin across the 3 DMA-capable queues so each
                # tensor lands roughly when its first consumer runs ----
                x_t = [[None] * 4 for _ in range(NQ)]
                wq_p, wk_p, wv_p = [], [], []
                bqk_sb = consts.tile([128, 2, NH], f32, name="bqk_sb")
                alibi_sb = consts.tile([128, NH * 16], f32, name="alibi_sb")
                masktri_sb = consts.tile([128, 128], bf16, name="masktri_sb")
                ones_sb = consts.tile([128, 128], r32, name="ones_sb")
                onesr_sb = consts.tile([1, 128], r32, name="onesr_sb")
                identb_sb = consts.tile([128, 128], bf16, name="identb_sb")
                tri01_sb = consts.tile([128, 128], bf16, name="tri01_sb")
                msh_sb = consts.tile([1, NH * QT], r32, name="msh_sb")
                wo_sb = wslab.tile([128, NH, T], bf16, name="wo_sb")
                # part g=0 of x chunk 0 and of wq lives in two half
                # tiles so the first matmuls wait on a ~0.7us DMA pair,
                # not the whole 512KB parts (readers wait on all writers
                # of a tile, so sub-tile DMA splits don't help)
                x0h = [wslab.tile([128, 1, QT], bf16, name=f"x0h{i}")
                       for i in range(4)]
                wq0h = [wslab.tile([128, 1, HG], bf16, name=f"wq0h{i}")
                        for i in range(4)]
                for g in range(4):
                    if g > 0:
                        x_t[0][g] = xp.tile([128, 4, QT], bf16,
                                            name=f"x_p{g}", tag=f"x_p{g}")
                    x_t[1][g] = xp.tile([128, 4, QT], bf16, name=f"x_p{g}",
                                        tag=f"x_p{g}")
                    wq_p.append(None if g == 0 else
                                wslab.tile([128, 4, HG], bf16,
                                           name=f"wq_p{g}"))
                    wk_p.append(wslab.tile([128, 4, HG], bf16,
                                           name=f"wk_p{g}"))
                    wv_p.append(wslab.tile([128, 4, HG], bf16,
                                           name=f"wv_p{g}"))

                def x0_sl(e, cols=slice(None)):
                    if e < 4:
                        return x0h[e][:, 0, cols]
                    return x_t[0][e // 4][:, e % 4, cols]

                def wq_sl(e, cols):
                    if e < 4:
                        return wq0h[e][:, 0, cols]
                    return wq_p[e // 4][:, e % 4, cols]

                wo_r = wo_d.ap().rearrange("(h p) o -> p h o", p=128)
                dmas = []  # (dst, src) in first-use order
                for g in range(4):
                    if g == 0:
                        for e in range(4):
                            dmas.append((x0h[e][:],
                                         xq_src(0, 0)[:, e:e + 1, :]))
                            dmas.append((wq0h[e][:],
                                         w_src(wq_d, 0)[:, e:e + 1, :]))
                            if e == 0:
                                dmas.append((bqk_sb[:], bqk_d.ap()))
                        continue
                    dmas.append((x_t[0][g][:], xq_src(0, g)))
                    dmas.append((wq_p[g][:], w_src(wq_d, g)))
                for g in range(4):
                    dmas.append((wk_p[g][:], w_src(wk_d, g)))
                dmas += [(alibi_sb[:], alibi_d.ap()),
                         (masktri_sb[:], masktri_d.ap()),
                         (ones_sb[:], ones_d.ap()),
                         (onesr_sb[:], onesrow_d.ap()),
                         (identb_sb[:], identb_d.ap()),
                         (tri01_sb[:], tri01_d.ap()),
                         (msh_sb[:], mshift_d.ap())]
                for g in range(4):
                    dmas.append((wv_p[g][:], w_src(wv_d, g)))
                # x chunk 1 prefetch (needed once proj(1) filler chains
                # start, well into attn(0)); wo only at the oproj(0)
                # fillers during attn(1)
                for g in range(4):
                    dmas.append((x_t[1][g][:], xq_src(1, g)))
                for h in range(NH):
                    dmas.append((wo_sb[:, h, :], wo_r[:, h, :]))
                qs = (nc.sync, nc.scalar, nc.gpsimd)
                for i, (dst, src) in enumerate(dmas):
                    qs[i % 3].dma_start(dst, src)

                qT_sb = [qkvp.tile([128, NH, QT], bf16, name=f"qT_sb{q}")
                         for q in range(NQ)]
                kT_sb = [qkvp.tile([128, NH, QT], bf16, name=f"kT_sb{q}")
                         for q in range(NQ)]
                v_sb = [qkvp.tile([128, 4, HG], bf16, name=f"v_sb{q}")
                        for q in range(NQ)]

                # ---- PE filler machinery: projection / output chains as
                # generators, one matmul emitted per pull ----
                def gen_qk_chain(c, pi, cc):
                    dst = (qT_sb, kT_sb)[pi]
                    csl = slice(cc * 128, (cc + 1) * 128)
                    ps = ps_p1.tile([128, 512], f32, name="p1acc",
                                    tag="p1acc")
                    for e in range(NE):
                        w_ap = (wq_sl(e, csl) if pi == 0 else
                                wk_p[e // 4][:, e % 4, csl])
                        nc.tensor.matmul(
                            ps[:], w_ap,
                            x_t[c][e // 4][:, e % 4, :],
                            start=(e == 0), stop=(e == NE - 1))
                        if e < NE - 1:
                            yield
                    nc.scalar.activation(
                        dst[c][:, cc, :], ps[:], Ident,
                        bias=bqk_sb[:, pi, cc:cc + 1])
                    yield

                def gen_v_chain(c, tb):
                    ps = ps_p1.tile([128, 512], f32, name="p1acc",
                                    tag="p1acc")
                    for e in range(NE):
                        nc.tensor.matmul(
                            ps[:],
                            x_t[c][e // 4][:, e % 4, tb * 128:(tb + 1) * 128],
                            wv_p[e // 4][:, e % 4, :],
                            start=(e == 0), stop=(e == NE - 1))
                        if e < NE - 1:
                            yield
                    nc.vector.tensor_copy(v_sb[c][:, tb, :], ps[:])
                    yield

                ys_pairs = {}

                def gen_oproj_chain(c, oc, outf_c):
                    if c == NQ - 1:
                        # final drain: attention is done, so the score /
                        # out PSUM pools and all copy engines are free —
                        # rotate across them so the 4-matmul chains never
                        # wait on a PSUM->SBUF copy
                        pool, ptag = [(ps_p1, "p1acc"), (ps_s, "s_ps"),
                                      (ps_o, "out_ps")][oc % 3]
                    else:
                        pool, ptag = ps_p1, "p1acc"
                    yp = pool.tile([128, 512], f32, name="y_ps", tag=ptag)
                    hord = (3, 2, 1, 0) if c == NQ - 1 else range(NH)
                    for hi, h in enumerate(hord):
                        nc.tensor.matmul(
                            yp[:],
                            wo_sb[:, h, oc * 128:(oc + 1) * 128],
                            outf_c[h][:],
                            start=(hi == 0), stop=(hi == 3))
                        if hi < NH - 1:
                            yield
                    # adjacent oc share one [128, 1024] SBUF tile so the
                    # store is a single 2KB-per-partition-line DMA
                    if oc % 2 == 0:
                        ys = ysb.tile([128, 1024], bf16, name="y_sb",
                                      tag="y_sb")
                        ys_pairs[c] = ys
                        nc.vector.tensor_copy(ys[:, 0:512], yp[:])
                        yield
                        return
                    ys = ys_pairs.pop(c)
                    nc.scalar.copy(ys[:, 512:1024], yp[:])
                    blk = (c * 8 + oc // 2) * 128
                    dst = yT_d.ap()[blk:blk + 128, :]
                    qeng = ([nc.gpsimd, nc.scalar, nc.sync][(oc // 2) % 3]
                            if c == NQ - 1 else
                            [nc.gpsimd, nc.scalar][(oc // 2) % 2])
                    qeng.dma_start(dst, ys[:])
                    yield

                filler = deque()

                def pull(n):
                    while n > 0 and filler:
                        try:
                            next(filler[0])
                            n -= 1
                        except StopIteration:
                            filler.popleft()

                def drain():
                    while filler:
                        try:
                            next(filler[0])
                        except StopIteration:
                            filler.popleft()

                def enqueue_proj(c):
                    for cc in range(4):
                        filler.append(gen_qk_chain(c, 0, cc))
                    for cc in range(4):
                        filler.append(gen_qk_chain(c, 1, cc))
                    for tb in range(4):
                        filler.append(gen_v_chain(c, tb))

                # ============ proj(0): standalone ============
                # Part-major with 4 parallel PSUM accumulators (the
                # score/out/den pools are all free at startup), so the PE
                # consumes each x/w part as its DMA lands instead of
                # serializing whole chains behind the 2-buffer p1 pool.
                def proj0_phase(accs, stat_of, mov_of, emit_copy):
                    for e in range(NE):
                        for j in range(4):
                            nc.tensor.matmul(
                                accs[j][:], stat_of(e, j), mov_of(e, j),
                                start=(e == 0), stop=(e == NE - 1))
                    for j in range(4):
                        emit_copy(j, accs[j])

                qk_accs = lambda: [
                    ps_p1.tile([128, 512], f32, name="p1acc", tag="p1acc"),
                    ps_p1.tile([128, 512], f32, name="p1acc", tag="p1acc"),
                    ps_s.tile([128, 512], f32, name="s_ps", tag="s_ps"),
                    ps_s.tile([128, 512], f32, name="s_ps", tag="s_ps"),
                ]
                proj0_phase(
                    qk_accs(),
                    lambda e, cc: wq_sl(e, slice(cc * 128, (cc + 1) * 128)),
                    lambda e, cc: x0_sl(e),
                    lambda cc, ps: nc.scalar.activation(
                        qT_sb[0][:, cc, :], ps[:], Ident,
                        bias=bqk_sb[:, 0, cc:cc + 1]))
                proj0_phase(
                    [ps_o.tile([128, 512], f32, name="out_ps", tag="out_ps"),
                     ps_o.tile([128, 512], f32, name="out_ps", tag="out_ps"),
                     ps_s.tile([128, 512], f32, name="s_ps", tag="s_ps"),
                     ps_d.tile([128, 512], f32, name="den_ps", tag="den_ps")],
                    lambda e, cc: wk_p[e // 4][:, e % 4,
                                               cc * 128:(cc + 1) * 128],
                    lambda e, cc: x0_sl(e),
                    lambda cc, ps: nc.scalar.activation(
                        kT_sb[0][:, cc, :], ps[:], Ident,
                        bias=bqk_sb[:, 1, cc:cc + 1]))
                proj0_phase(
                    qk_accs(),
                    lambda e, tb: x0_sl(e, slice(tb * 128, (tb + 1) * 128)),
                    lambda e, tb: wv_p[e // 4][:, e % 4, :],
                    lambda tb, ps: nc.vector.tensor_copy(
                        v_sb[0][:, tb, :], ps[:]))

                for c in range(NQ):
                    # x for chunk c+2 loads during attn(c) (x1 was loaded
                    # at startup); needed by proj(c+2) fillers in attn(c+1)
                    if 2 <= c + 2 < NQ:
                        for g in range(4):
                            x_t[c + 2][g] = xp.tile(
                                [128, 4, QT], bf16,
                                name=f"x_p{g}", tag=f"x_p{g}")
                            nc.sync.dma_start(
                                x_t[c + 2][g][:], xq_src(c + 2, g))

                    # proj(c+1) chains become filler for attn(c), behind
                    # any oproj(c-1) chains still queued
                    if c + 1 < NQ:
                        enqueue_proj(c + 1)

                    # ================ attn(c) ================
                    # per-head outf tiles keep oproj from waiting on the
                    # later heads' normalization tails
                    outf_c = [outfp.tile([128, 512], bf16,
                                         name=f"outf_h{h}", tag=f"outf_h{h}")
                              for h in range(NH)]
                    # last chunk: biggest head first so oproj(NQ-1) chains
                    # (accumulated in the same order) start while the
                    # small heads' softmax tails are still normalizing
                    horder = (3, 2, 1, 0) if c == NQ - 1 else range(NH)
                    for h in horder:
                        msh = needs_msh(c, h)
                        kept = list(kept_range(c, h))
                        first, last = kept[0], kept[-1]

                        s_tiles = {}
                        p_tiles = {}

                        def qlo_of(jc):
                            # queries below 128*(jc-4c) see no valid key
                            # in diag tile jc: skip that region entirely
                            return max(0, (jc - 4 * c) * 128)

                        def emit_score(jc):
                            # causal mask (one 128x128 triangle) and the
                            # rank-1 stabilization shift accumulate onto
                            # the score PSUM on the PE itself.
                            # The start matmul is always full-width so the
                            # PSUM bank has a uniform accumulation state.
                            s = ps_s.tile([128, 512], f32, name="s_ps",
                                          tag="s_ps")
                            diag = jc >= 4 * c
                            dlo = qlo_of(jc)
                            slo = 0 if sim_safe else dlo
                            mask_mm = diag and msh
                            nc.tensor.matmul(
                                s[:, slo:],
                                kT_sb[jc // 4][:, h,
                                               (jc % 4) * 128:
                                               (jc % 4 + 1) * 128],
                                qT_sb[c][:, h, slo:],
                                start=True, stop=not (mask_mm or msh))
                            if mask_mm:
                                nc.tensor.matmul(
                                    s[:, dlo:dlo + 128],
                                    identb_sb[:], masktri_sb[:],
                                    start=False, stop=not msh)
                            if msh:
                                mlo = min(dlo, 256)
                                nc.tensor.matmul(
                                    s[:, mlo:],
                                    onesr_sb[:],
                                    msh_sb[0:1, h * QT + mlo:
                                           (h + 1) * QT],
                                    start=False, stop=True)
                            s_tiles[jc] = s

                        def emit_exp(jc):
                            qlo = qlo_of(jc)
                            p = pp.tile([128, 512], bf16, name="p_sb",
                                        tag="p_sb")
                            nc.scalar.activation(
                                p[:, qlo:], s_tiles.pop(jc)[:, qlo:], Exp,
                                bias=alibi_sb[:, h * 16 + 4 * c - jc + 3:
                                              h * 16 + 4 * c - jc + 4])
                            if jc >= 4 * c and not msh:
                                # causal mask applied on p just ahead of
                                # the pacc add in the DVE FIFO; saves the
                                # PSUM mask matmul on the PE
                                nc.vector.tensor_tensor(
                                    p[:, qlo:qlo + 128],
                                    p[:, qlo:qlo + 128], tri01_sb[:],
                                    mybir.AluOpType.mult)
                            p_tiles[jc] = p

                        outp = ps_o.tile([128, 512], f32, name="out_ps",
                                         tag="out_ps")
                        # p accumulates on DVE (SBUF only); den is one
                        # matmul per head instead of one per tile
                        pacc = paccp.tile([128, 512], r32, name="pacc_sb",
                                          tag="pacc_sb")

                        def emit_consume(jc):
                            p = p_tiles.pop(jc)
                            qlo = qlo_of(jc)
                            if jc == first:
                                nc.vector.tensor_copy(pacc[:], p[:])
                            else:
                                nc.vector.tensor_tensor(
                                    pacc[:, qlo:], pacc[:, qlo:],
                                    p[:, qlo:], AddOp)
                            nc.tensor.matmul(
                                outp[:, qlo:],
                                v_sb[jc // 4][:, jc % 4,
                                              h * 128:(h + 1) * 128],
                                p[:, qlo:],
                                start=(jc == first), stop=(jc == last))

                        for i in range(min(LOOK, len(kept))):
                            emit_score(kept[i])
                        for i, jc in enumerate(kept):
                            if i + LOOK < len(kept):
                                emit_score(kept[i + LOOK])
                            pull(2)
                            emit_exp(jc)
                            emit_consume(jc)

                        # filler before den hides the DVE pacc tail
                        pull(3)
                        den = ps_d.tile([128, 512], f32, name="den_ps",
                                        tag="den_ps")
                        nc.tensor.matmul(den[:], ones_sb[:], pacc[:],
                                         start=True, stop=True)
                        rcp = rcpp.tile([128, 512], f32, name="rcp",
                                        tag="rcp")
                        with nc.allow_low_precision(
                                reason="elementwise reciprocal"):
                            nc.vector.reciprocal(rcp[:], den[:])
                        nc.vector.tensor_mul(
                            outf_c[h][:], outp[:], rcp[:])
                        pull(2)

                    # finish oproj(c-1) + proj(c+1) chains before oproj(c)
                    drain()

                    # oproj(c) chains: filler for attn(c+1) (or drained
                    # at the end for the last chunk)
                    for oc in range(16):
                        filler.append(gen_oproj_chain(c, oc, outf_c))

                drain()

            if reps == 1:
                body()
            else:
                # unroll the rep loop: each For_i iteration carries an
                # all-engine barrier, so amortize it over several bodies
                u = 1
                for cand in (4, 3, 2):
                    if reps % cand == 0:
                        u = cand
                        break
                with tc.For_i(0, reps // u, 1):
                    for _ in range(u):
                        body()

    nc.compile()
    return nc


def get_slopes():
    start = 2 ** (-2 ** (-(math.log2(16) - 3)))
    return np.array([start * start ** i for i in range(16)], np.float32)


def make_host_inputs(x, Wq, bq, Wk, bk, Wv, bv, Wo, bo):
    """Shard full inputs into 8 per-core input maps."""
    nbf16 = mybir.dt.np(bf16)
    x = np.asarray(x, np.float32)
    Wq = np.asarray(Wq, np.float32); bq = np.asarray(bq, np.float32)
    Wk = np.asarray(Wk, np.float32); bk = np.asarray(bk, np.float32)
    Wv = np.asarray(Wv, np.float32)
    Wo = np.asarray(Wo, np.float32)

    slopes = get_slopes()
    sc = np.float32(1.0 / math.sqrt(HD))
    jl = np.arange(128, dtype=np.float32)
    il = np.arange(QT, dtype=np.float32)

    masktri = np.where(jl[:, None] > np.arange(128)[None, :],
                       np.float32(-1e10), np.float32(0.0))
    tri01 = np.where(jl[:, None] > np.arange(128)[None, :],
                     np.float32(0.0), np.float32(1.0))
    ones128 = np.ones((128, 128), np.float32)
    onesrow = np.ones((1, 128), np.float32)
    identb = np.eye(128, dtype=np.float32)

    in_maps = []
    for core in range(8):
        b, s = core // 4, core % 4
        heads = [s, s + 4, s + 8, s + 12]
        cols = np.concatenate(
            [np.arange(h * HD, (h + 1) * HD) for h in heads])
        # bias column for tile (c, jc): anchored at the last query of
        # chunk c:  b[jl] = -slope * (128*(4c - jc) + 511 - jl)
        alibi = np.empty((128, NH * 16), np.float32)
        mshift = np.empty((1, NH * QT), np.float32)
        for hh, h in enumerate(heads):
            anchor = 256.0 if hh == 1 else 511.0
            for dd in range(16):
                kk = dd - 3  # 4c - jc
                alibi[:, hh * 16 + dd] = -slopes[h] * (
                    128.0 * kk + anchor - jl)
            mshift[0, hh * QT:(hh + 1) * QT] = slopes[h] * (511.0 - il)
        bqk = np.zeros((128, 2, NH), np.float32)
        bqk[:, 0, :] = (bq[cols] * sc).reshape(NH, HD).T
        bqk[:, 1, :] = bk[cols].reshape(NH, HD).T
        def pack_x(xb):
            # [emb, t] -> tile-packed [(q*4+g)*128+p, c, t]
            a = xb.reshape(4, 4, 128, 4, 512)          # [g, c, p, q, t]
            return np.ascontiguousarray(
                a.transpose(3, 0, 2, 1, 4).reshape(16 * 128, 4, 512))

        def pack_w(w):
            # [emb, m] -> [(g*128+p), c, m]
            a = w.reshape(4, 4, 128, HG)               # [g, c, p, m]
            return np.ascontiguousarray(
                a.transpose(0, 2, 1, 3).reshape(4 * 128, 4, HG))

        in_maps.append({
            "xq": pack_x(x[b].T).astype(nbf16),
            "wq": pack_w(Wq[:, cols] * sc).astype(nbf16),
            "wk": pack_w(Wk[:, cols]).astype(nbf16),
            "wv": pack_w(Wv[:, cols]).astype(nbf16),
            "wo": np.ascontiguousarray(Wo[cols, :]).astype(nbf16),
            "bqk": bqk,
            "alibi": alibi,
            "mshift": mshift,
            "masktri": masktri.astype(nbf16),
            "ones": ones128,
            "onesrow": onesrow,
            "identb": identb.astype(nbf16),
            "tri01": tri01.astype(nbf16),
        })
    return in_maps


def assemble_output(results, Wv_bias=None, bo=None, Wo=None):
    """results: list of 8 per-core dicts with 'yT' (bf16 partials).

    v-bias folds out of attention exactly: out_h = attn(v'_h) + bv_h,
    so y = sum_h out_h Wo_h = y' + bv @ Wo. Added here with bo.
    Positional-compat: assemble_output(results, bo) treats bv as zero.
    """
    if bo is None:
        Wv_bias, bo = None, Wv_bias
    bo = np.asarray(bo, np.float32)
    if Wv_bias is not None and Wo is not None and np.any(Wv_bias):
        extra = np.asarray(Wv_bias, np.float32) @ np.asarray(Wo, np.float32) + bo
    else:
        extra = bo  # (2048,)
    out = np.empty((2, T, EMB), np.float32)
    for b in range(2):
        acc = np.asarray(results[b * 4 + 0]["yT"], np.float32)
        for s in range(1, 4):
            acc += np.asarray(results[b * 4 + s]["yT"], np.float32)
        # un-pack the pairwise tile layout [c*8+j, 128, 2, 512] back to
        # [emb_out, tok]
        acc = acc.reshape(4, 8, 128, 2, 512).transpose(
            1, 3, 2, 0, 4).reshape(T, T)
        out[b] = acc.T + extra
    return out


class SpmdRunner:
    def __init__(self, nc, n_cores: int):
        install_neuronx_cc_hook()
        self.nc = nc
        self.n_cores = n_cores
        assert nc.dbg_addr is None or not nc.dbg_callbacks
        partition_name = (
            nc.partition_id_tensor.name if nc.partition_id_tensor else None
        )
        in_names, out_names, out_avals = [], [], []
        for alloc in nc.m.functions[0].allocations:
            if not isinstance(alloc, mybir.MemoryLocationSet):
                continue
            name = alloc.memorylocations[0].name
            if alloc.kind == "ExternalInput":
                if name != partition_name:
                    in_names.append(name)
            elif alloc.kind == "ExternalOutput":
                shape = tuple(alloc.tensor_shape)
                dtype = mybir.dt.np(alloc.dtype)
                out_names.append(name)
                out_avals.append(jax.core.ShapedArray(shape, dtype))
        self.in_names = list(in_names)
        self.out_names = out_names
        self.out_avals = out_avals
        n_params = len(self.in_names)
        all_in_names = list(in_names) + list(out_names)
        if partition_name is not None:
            all_in_names.append(partition_name)
        self.partition_name = partition_name

        def _body(*args):
            operands = list(args)
            if partition_name is not None:
                operands.append(bass2jax.partition_id_tensor())
            outs = _bass_exec_p.bind(
                *operands,
                out_avals=tuple(out_avals),
                in_names=tuple(all_in_names),
                out_names=tuple(out_names),
                lowering_input_output_aliases=(),
                sim_require_finite=True,
                sim_require_nnan=True,
                nc=nc,
            )
            return tuple(outs)

        devices = jax.devices()[:n_cores]
        assert len(devices) == n_cores
        self.mesh = Mesh(np.asarray(devices), ("core",))
        n_outs = len(out_names)
        in_specs = (PartitionSpec("core"),) * (n_params + n_outs)
        out_specs = (PartitionSpec("core"),) * n_outs
        self.fn = jax.jit(
            shard_map(_body, mesh=self.mesh, in_specs=in_specs,
                      out_specs=out_specs, check_rep=False),
            keep_unused=True,
        )
        self.dev_args = None

    def set_inputs(self, in_maps: list[dict]):
        """device_put concatenated per-core inputs + zero output buffers."""
        n = self.n_cores
        assert len(in_maps) == n
        concat_in = [
            np.concatenate([np.asarray(in_maps[c][name]) for c in range(n)], axis=0)
            for name in self.in_names
        ]
        concat_zeros = [
            np.zeros((n * a.shape[0], *a.shape[1:]), a.dtype) for a in self.out_avals
        ]
        sharding = jax.sharding.NamedSharding(self.mesh, PartitionSpec("core"))
        self.dev_args = [jax.device_put(a, sharding) for a in concat_in + concat_zeros]

    def run(self):
        outs = self.fn(*self.dev_args)
        jax.block_until_ready(outs)
        return outs

    def results(self, outs) -> list[dict]:
        n = self.n_cores
        return [
            {
                name: np.asarray(outs[i]).reshape(n, *self.out_avals[i].shape)[c]
                for i, name in enumerate(self.out_names)
            }
            for c in range(n)
        ]

    def time_execs(self, iters: int = 10, warmup: int = 2):
        for _ in range(warmup):
            self.run()
        t0 = time.perf_counter()
        for _ in range(iters):
            outs = self.fn(*self.dev_args)
        jax.block_until_ready(outs)
        t1 = time.perf_counter()
        return (t1 - t0) / iters


_RUNNER = None


def _get_runner():
    global _RUNNER
    if _RUNNER is None:
        nc = build_program(reps=1)
        _RUNNER = SpmdRunner(nc, 8)
    return _RUNNER


def kernel(x, Wq, bq, Wk, bk, Wv, bv, Wo, bo):
    r = _get_runner()
    in_maps = make_host_inputs(x, Wq, bq, Wk, bk, Wv, bv, Wo, bo)
    r.set_inputs(in_maps)
    outs = r.run()
    res = r.results(outs)
    return assemble_output(res, bv, bo, Wo)
